# revision 1
# baseline (speedup 1.0000x reference)
"""DeepSeek sparse attention TRN2 kernel: 8-core query-parallel.

Hardcoded for B=1, S=768, E=512, H=8, DK=64, TOPK=384, 8 cores.
  - Core c owns queries [96c, 96c+96). Output = host concat of per-core rows.
  - Indexer chain in fp32 matmuls (top-k set needs ~1e-5 score accuracy).
  - Top-k via per-row threshold: 16 bisection steps with ACT Sign+accum
    counting, then exact top-16 fixup (max8 + match_replace + max8).
    Tie-break ramp -t*2^-40 reproduces lax.top_k's lower-index-first
    ordering on the exact-zero relu atom.
  - Attention = dense QK^T + multiplicative 0/1 mask (math-identical to
    gather+softmax over the selected set), bf16.
  - bk dropped (softmax shift-invariance); bv folded into bo2 on host.
"""
import numpy as np
import ml_dtypes

S, E, H, DK = 768, 512, 8, 64
NQ = 96
NC = 8
KCH = 4            # 512/128
TCH = 6            # 768/128
TH = 384           # t-half for fp32 PSUM-bank-sized N
SCALING = 1.0 / np.sqrt(DK)
RAMP_EPS = float(2.0 ** -40)
R_ITERS = 8
BRK = 1.2
NEG = -1e30


def build_nc(stage=99):
    import concourse.bass as bass
    import concourse.bacc as bacc
    from concourse import mybir
    from concourse.tile import TileContext

    f32 = mybir.dt.float32
    bf16 = mybir.dt.bfloat16
    AF = mybir.ActivationFunctionType
    OP = mybir.AluOpType

    nc = bacc.Bacc("TRN2", target_bir_lowering=False, debug=False)

    def din(name, shape, dt):
        return nc.dram_tensor(name, shape, dt, kind="ExternalInput")

    xT = din("xT", [E, S], f32)
    iqW = din("iqW", [E, E], f32)
    ikW = din("ikW", [E, DK], f32)
    wpW = din("wpW", [E, H], f32)
    wq16 = din("wq16", [E, E], bf16)
    wk16 = din("wk16", [E, E], bf16)
    wv16 = din("wv16", [E, E], bf16)
    wo16 = din("wo16", [DK, H, E], bf16)
    iqb = din("iqb", [E, 1], f32)
    ikb = din("ikb", [DK, 1], f32)
    wpb = din("wpb", [H, 1], f32)
    bqh = din("bqh", [DK, H], f32)
    bo2 = din("bo2", [1, E], f32)
    bd01 = din("bd01", [128, 160], f32)
    nramp = din("nramp", [1, S], f32)
    ones96 = din("ones96", [1, NQ], f32)
    col16 = din("col16", [1, 16], f32)
    xTq = din("xTq", [E, NQ], f32)
    out = nc.dram_tensor("out", [NQ, E], f32, kind="ExternalOutput")
    dbg = nc.dram_tensor("dbg", [NQ, S], f32, kind="ExternalOutput")
    wT_dram = nc.dram_tensor("wT_dram", [H, NQ], f32)
    den_dram = nc.dram_tensor("den_dram", [2, 4 * NQ], f32)

    def bcastP(ap, p):
        return bass.AP(tensor=ap.tensor, offset=ap.offset,
                       ap=[[0, p]] + ap.ap[1:])

    import contextlib
    with TileContext(nc) as tc:
      with contextlib.suppress(StopIteration):
        with tc.tile_pool(name="w1", bufs=1) as w1, \
             tc.tile_pool(name="big", bufs=1) as big, \
             tc.tile_pool(name="scp", bufs=2) as scp, \
             tc.tile_pool(name="tiny", bufs=1) as tiny, \
             tc.tile_pool(name="psA", bufs=3, space="PSUM") as psA, \
             tc.tile_pool(name="psB", bufs=1, space="PSUM") as psB:

            # ---------------- loads (chunked [128, k, n]) ----------------
            s_xT = w1.tile([128, KCH, S], f32)
            s_xT16 = w1.tile([128, KCH, S], bf16)
            s_xTq = w1.tile([128, KCH, NQ], f32)
            s_xTq16 = w1.tile([128, KCH, NQ], bf16)
            s_iqW = w1.tile([128, KCH, E], f32)
            s_ikW = w1.tile([128, KCH, DK], f32)
            s_wpW = w1.tile([128, KCH, H], f32)
            s_wq = w1.tile([128, KCH, E], bf16)
            s_wk = w1.tile([128, KCH, E], bf16)
            s_wv = w1.tile([128, KCH, E], bf16)
            s_wo = w1.tile([DK, H, E], bf16)
            s_iqb = w1.tile([128, KCH], f32)
            s_bqh = w1.tile([DK, H], f32)
            s_ikb = w1.tile([DK, 1], f32)
            s_wpb = w1.tile([H, 1], f32)
            s_bd01 = w1.tile([128, 160], f32)
            s_nramp = w1.tile([1, S], f32)
            s_ones96 = w1.tile([1, NQ], f32)
            s_col16 = w1.tile([NQ, 16], f32)
            s_bo2 = w1.tile([NQ, E], f32)

            for dst, src in [(s_ikW, ikW), (s_xT, xT), (s_xTq, xTq),
                             (s_iqW, iqW), (s_wpW, wpW), (s_wq, wq16), (s_wk, wk16),
                             (s_wv, wv16)]:
                nc.sync.dma_start(
                    out=dst, in_=src[:, :].rearrange("(k p) n -> p k n", p=128))
            nc.sync.dma_start(
                out=s_iqb, in_=iqb[:, :].rearrange("(k p) o -> p (k o)", p=128))
            nc.sync.dma_start(out=s_wo, in_=wo16[:, :, :])
            nc.vector.tensor_copy(s_xT16.rearrange("p k n -> p (k n)"),
                                  s_xT.rearrange("p k n -> p (k n)"))
            nc.vector.tensor_copy(s_xTq16.rearrange("p k n -> p (k n)"),
                                  s_xTq.rearrange("p k n -> p (k n)"))
            nc.sync.dma_start(out=s_bqh, in_=bqh[:, :])
            nc.sync.dma_start(out=s_ikb, in_=ikb[:, :])
            nc.sync.dma_start(out=s_wpb, in_=wpb[:, :])
            nc.sync.dma_start(out=s_bd01, in_=bd01[:, :])
            nc.sync.dma_start(out=s_nramp, in_=nramp[:, :])
            nc.sync.dma_start(out=s_ones96, in_=ones96[:, :])
            nc.sync.dma_start(out=s_col16, in_=bcastP(col16[:, :], NQ))
            nc.sync.dma_start(out=s_bo2, in_=bcastP(bo2[:, :], NQ))

            if stage == 11:
                s_oA = big.tile([NQ, E], f32, name="s_oA")
                nc.vector.tensor_copy(s_oA, s_bo2)
                nc.vector.tensor_copy(s_oA[:, 0:1], s_xT[:96, 0, 0:1])
                nc.sync.dma_start(out=out[:, :], in_=s_oA)
                raise StopIteration
            # =========== INDEXER (fp32) ===========
            s_kidT = big.tile([DK, S], f32)
            for th in range(2):
                pk = psA.tile([DK, TH], f32, tag="ps")
                for k in range(KCH):
                    nc.tensor.matmul(pk, s_ikW[:, k, :],
                                     s_xT[:, k, TH * th:TH * (th + 1)],
                                     start=(k == 0), stop=(k == KCH - 1))
                nc.scalar.activation(out=s_kidT[:, TH * th:TH * (th + 1)],
                                     in_=pk, func=AF.Identity, bias=s_ikb)

            s_qidT = big.tile([128, KCH, NQ], f32)
            for m in range(KCH):
                pq = psA.tile([128, NQ], f32, tag="ps")
                for k in range(KCH):
                    nc.tensor.matmul(pq, s_iqW[:, k, 128 * m:128 * (m + 1)],
                                     s_xTq[:, k, :],
                                     start=(k == 0), stop=(k == KCH - 1))
                nc.scalar.activation(out=s_qidT[:, m, :], in_=pq,
                                     func=AF.Identity,
                                     bias=s_iqb[:, m:m + 1])

            s_widT = tiny.tile([H, NQ], f32)
            pw = psA.tile([H, NQ], f32, tag="ps")
            for k in range(KCH):
                nc.tensor.matmul(pw, s_wpW[:, k, :], s_xTq[:, k, :],
                                 start=(k == 0), stop=(k == KCH - 1))
            nc.scalar.activation(out=s_widT, in_=pw, func=AF.Identity,
                                 bias=s_wpb)
            nc.sync.dma_start(out=wT_dram[:, :], in_=s_widT)

            if stage == 12:
                s_oB = big.tile([NQ, E], f32, name="s_oB")
                nc.vector.tensor_copy(s_oB, s_bo2)
                nc.vector.tensor_copy(s_oB[:, 0:1], s_kidT[:32, 0:1].to_broadcast([32, 1]))
                nc.vector.tensor_copy(s_oB[:, 1:2], s_qidT[:96, 0, 0:1])
                nc.sync.dma_start(out=out[:, :], in_=s_oB)
                raise StopIteration
            # score lhsT tiles [64, 128]: col = 32*hl + s  (hl-major)
            # rows d; head h = 4*hf + hl; queries s in group g (32 wide)
            sc_lhs = [[tiny.tile([DK, 128], f32, tag=f"sclhs_{g}_{hf}", name=f"sclhs_{g}_{hf}")
                       for hf in range(2)] for g in range(3)]
            for g in range(3):
                for hf in range(2):
                    for r in (0, DK):
                        # hl = {0,2} (r=0) or {1,3} (r=64): chunks m = 2*hf, 2*hf+1
                        sl = sc_lhs[g][hf]
                        dst = bass.AP(
                            tensor=sl.tensor,
                            offset=sl.offset + 32 * (r // DK),
                            ap=[sl.ap[0], [64, 2], [1, 32]])
                        nc.sync.dma_start(
                            out=dst,
                            in_=s_qidT[r:r + DK, 2 * hf:2 * hf + 2,
                                       32 * g:32 * (g + 1)])

            # w columns [128,1]: partition 32*hl+s -> w[32g+s, 4hf+hl]
            w_cols = [[tiny.tile([128, 1], f32, tag=f"wcol_{g}_{hf}", name=f"wcol_{g}_{hf}")
                       for hf in range(2)] for g in range(3)]
            for g in range(3):
                for hf in range(2):
                    for hl in range(4):
                        nc.sync.dma_start(
                            out=w_cols[g][hf][32 * hl:32 * (hl + 1), :],
                            in_=wT_dram[4 * hf + hl:4 * hf + hl + 1,
                                        32 * g:32 * (g + 1)])
            # scores + relu*w
            ws = [[[scp.tile([128, TH], f32, tag=f"ws_{g}_{hf}_{th}", name=f"ws_{g}_{hf}_{th}")
                    for th in range(2)] for hf in range(2)] for g in range(3)]
            for g in range(3):
                for hf in range(2):
                    for th in range(2):
                        psc = psA.tile([128, TH], f32, tag="ps")
                        nc.tensor.matmul(psc, sc_lhs[g][hf],
                                         s_kidT[:, TH * th:TH * (th + 1)],
                                         start=True, stop=True)
                        nc.vector.scalar_tensor_tensor(
                            out=ws[g][hf][th], in0=psc, scalar=0.0,
                            in1=w_cols[g][hf].to_broadcast([128, TH]),
                            op0=OP.max, op1=OP.mult)

            if stage == 13:
                s_oC = big.tile([NQ, E], f32, name="s_oC")
                nc.vector.tensor_copy(s_oC, s_bo2)
                nc.vector.tensor_copy(s_oC[:, 0:1], ws[0][0][0][:96, 0:1])
                nc.vector.tensor_copy(s_oC[:, 1:2], ws[2][1][1][:96, 0:1])
                nc.sync.dma_start(out=out[:, :], in_=s_oC)
                raise StopIteration
            # combine -> ind (with tie-break ramp subtracted)
            s_ind = big.tile([NQ, S], f32)
            for th in range(2):
                pind = psB.tile([NQ, TH], f32, tag="pind")
                first = True
                for g in range(3):
                    for hf in range(2):
                        nc.tensor.matmul(
                            pind, s_bd01[:, 64 - 32 * g:160 - 32 * g],
                            ws[g][hf][th], start=first, stop=False)
                        first = False
                nc.tensor.matmul(pind, s_ones96,
                                 s_nramp[:, TH * th:TH * (th + 1)],
                                 start=False, stop=True)
                nc.scalar.copy(s_ind[:, TH * th:TH * (th + 1)], pind)

            if stage < 90:
                nc.sync.dma_start(out=dbg[:, :], in_=s_ind)
            if stage < 2:
                s_o0 = big.tile([NQ, E], f32, name="s_o0")
                nc.vector.memset(s_o0, 0.0)
                nc.sync.dma_start(out=out[:, :], in_=s_o0)
                raise StopIteration
            # =========== TOPK threshold ===========
            lo = tiny.tile([NQ, 1], f32)
            hi = tiny.tile([NQ, 1], f32)
            tmp = tiny.tile([NQ, 1], f32)
            nmid = tiny.tile([NQ, 1], f32)
            mid = tiny.tile([NQ, 1], f32)
            u8 = mybir.dt.uint8
            cmp = tiny.tile([NQ, 1], u8)
            ncmp = tiny.tile([NQ, 1], u8)
            acc = tiny.tile([NQ, 1], f32)
            sgn_scr = big.tile([NQ, S], f32)
            rsum = tiny.tile([NQ, 1], f32, name="rsum")
            mscr = big.tile([NQ, S], f32, tag="mscr", name="mscr")
            nc.scalar.activation(out=mscr, in_=s_ind, func=AF.Identity,
                                 bias=0.0, accum_out=rsum)
            nc.vector.tensor_scalar(lo, rsum, 1.0 / S, -BRK, op0=OP.mult,
                                    op1=OP.add)
            nc.vector.tensor_scalar(hi, rsum, 1.0 / S, BRK, op0=OP.mult,
                                    op1=OP.add)
            cnt2 = tiny.tile([NQ, 1], f32)
            t2 = tiny.tile([NQ, 1], f32)
            u = tiny.tile([NQ, 1], f32)
            scr2 = big.tile([NQ, S - 384], bf16, tag="scr2")
            for r in range(R_ITERS):
                nc.vector.tensor_add(tmp, lo, hi)
                nc.vector.tensor_scalar_mul(nmid, tmp, -0.5)
                nc.vector.tensor_scalar_mul(mid, tmp, 0.5)
                # ACT counts cols [0,512); DVE counts [512,768)
                nc.scalar.activation(out=sgn_scr[:, :384],
                                     in_=s_ind[:, :384], func=AF.Sign,
                                     bias=nmid, scale=1.0, accum_out=acc)
                nc.vector.tensor_scalar(scr2, s_ind[:, 384:], mid, None,
                                        op0=OP.is_ge, op1=OP.add,
                                        accum_out=cnt2)
                nc.vector.tensor_scalar(t2, cnt2, 2.0, -384.0, op0=OP.mult,
                                        op1=OP.add)
                nc.vector.tensor_add(u, acc, t2)
                nc.vector.tensor_scalar(cmp, u, 0.0, None, op0=OP.is_ge)
                nc.vector.tensor_scalar(ncmp, u, 0.0, None, op0=OP.is_lt)
                nc.vector.copy_predicated(lo, cmp, mid)
                nc.vector.copy_predicated(hi, ncmp, mid)

            # exact count at hi; in-bracket top-16
            scr_b = big.tile([NQ, S], bf16, tag="scr_b")
            c_hi = tiny.tile([NQ, 1], f32)
            nc.vector.tensor_scalar(scr_b, s_ind, hi, None, op0=OP.is_ge,
                                    op1=OP.add, accum_out=c_hi)
            negbig = tiny.tile([NQ, 1], f32, name="negbig")
            nc.vector.memset(negbig, NEG)
            hicut = big.tile([NQ, S], f32, tag="hicut")
            nc.vector.scalar_tensor_tensor(
                out=hicut, in0=s_ind, scalar=hi,
                in1=negbig.to_broadcast([NQ, S]), op0=OP.is_ge, op1=OP.mult)
            mlo = big.tile([NQ, S], f32, tag="mlo")
            nc.vector.tensor_add(mlo, hicut, s_ind)
            m16 = tiny.tile([NQ, 16], f32)
            mlo2 = big.tile([NQ, S], f32, tag="mlo2")
            nc.vector.max(out=m16[:, 0:8], in_=mlo)
            nc.vector.match_replace(out=mlo2, in_to_replace=m16[:, 0:8],
                                    in_values=mlo, imm_value=NEG)
            nc.vector.max(out=m16[:, 8:16], in_=mlo2)
            need_m1 = tiny.tile([NQ, 1], f32)
            nc.vector.tensor_scalar(need_m1, c_hi, -1.0, 383.0, op0=OP.mult,
                                    op1=OP.add)
            oh = tiny.tile([NQ, 16], f32)
            oh2 = tiny.tile([NQ, 16], f32)
            tstar = tiny.tile([NQ, 1], f32)
            nc.vector.tensor_scalar(oh, s_col16, need_m1, None, op0=OP.is_equal)
            nc.vector.scalar_tensor_tensor(out=oh2, in0=m16, scalar=1.0,
                                           in1=oh, op0=OP.mult, op1=OP.mult,
                                           accum_out=tstar)
            mask01 = big.tile([NQ, S], bf16, tag="mask01")
            nc.vector.tensor_scalar(mask01, s_ind, tstar, None, op0=OP.is_ge)
            # transpose mask -> maskT [128, 6, 96]
            s_maskT = big.tile([128, TCH, NQ], bf16)
            for t in range(TCH):
                nc.sync.dma_start_transpose(
                    s_maskT[:, t, :], mask01[:, 128 * t:128 * (t + 1)])

            # =========== ATTENTION (bf16) ===========
            s_KT = big.tile([DK, H, S], bf16)
            s_QT = big.tile([DK, H, NQ], bf16)
            for h in range(H):
                for th in range(2):
                    pk2 = psA.tile([DK, TH], f32, tag="ps")
                    for k in range(KCH):
                        nc.tensor.matmul(pk2,
                                         s_wk[:, k, DK * h:DK * (h + 1)],
                                         s_xT16[:, k, TH * th:TH * (th + 1)],
                                         start=(k == 0), stop=(k == KCH - 1))
                    nc.scalar.copy(s_KT[:, h, TH * th:TH * (th + 1)], pk2)
                pq2 = psA.tile([DK, NQ], f32, tag="ps")
                for k in range(KCH):
                    nc.tensor.matmul(pq2, s_wq[:, k, DK * h:DK * (h + 1)],
                                     s_xTq16[:, k, :],
                                     start=(k == 0), stop=(k == KCH - 1))
                nc.scalar.activation(out=s_QT[:, h, :], in_=pq2,
                                     func=AF.Identity, bias=s_bqh[:, h:h + 1])
            s_V = big.tile([128, TCH, E], bf16)
            for t in range(TCH):
                pv = psA.tile([128, E], f32, tag="ps")
                for k in range(KCH):
                    nc.tensor.matmul(pv, s_xT16[:, k, 128 * t:128 * (t + 1)],
                                     s_wv[:, k, :],
                                     start=(k == 0), stop=(k == KCH - 1))
                nc.scalar.copy(s_V[:, t, :], pv)


            w_tiles = [[scp.tile([128, 4 * NQ], bf16, tag=f"wt_{t}_{q}", name=f"wt_{t}_{q}") for q in range(2)] for t in range(TCH)]
            for t in range(TCH):
                for q in range(2):
                    psc2 = psA.tile([128, 4 * NQ], f32, tag="ps")
                    for hl in range(4):
                        h = 4 * q + hl
                        nc.tensor.matmul(
                            psc2[:, NQ * hl:NQ * (hl + 1)],
                            s_KT[:, h, 128 * t:128 * (t + 1)],
                            s_QT[:, h, :],
                            start=True, stop=True)
                    nc.scalar.activation(out=w_tiles[t][q], in_=psc2,
                                         func=AF.Exp, scale=SCALING)
            pden = [psB.tile([1, 4 * NQ], f32, tag=f"pden{q}", name=f"pden{q}")
                    for q in range(2)]
            onesrow = tiny.tile([128, 1], bf16)
            nc.vector.memset(onesrow, 1.0)
            for t in range(TCH):
                msl = s_maskT[:, t, :]
                mrep = bass.AP(tensor=msl.tensor, offset=msl.offset,
                               ap=[msl.ap[0], [0, 4]] + msl.ap[1:])
                for q in range(2):
                    wt = w_tiles[t][q]
                    nc.vector.tensor_mul(wt, wt, mrep)
            for q in range(2):
                for t in range(TCH):
                    nc.tensor.matmul(pden[q], onesrow, w_tiles[t][q],
                                     start=(t == 0), stop=(t == TCH - 1))

            s_den = tiny.tile([1, 4 * NQ], f32)
            s_den2 = tiny.tile([1, 4 * NQ], f32)
            nc.vector.reciprocal(s_den, pden[0])
            nc.vector.reciprocal(s_den2, pden[1])
            nc.sync.dma_start(out=den_dram[0:1, :], in_=s_den)
            nc.sync.dma_start(out=den_dram[1:2, :], in_=s_den2)

            rbq = [tiny.tile([DK, 4 * NQ], f32, name=f"rbq{q}")
                   for q in range(2)]
            for q in range(2):
                nc.sync.dma_start(out=rbq[q],
                                  in_=bcastP(den_dram[q:q + 1, :], DK))
            s_attn = [big.tile([DK, NQ], bf16, tag=f"attn{h}", name=f"attn{h}")
                      for h in range(H)]
            for h in range(H):
                half = h % 2
                pa = psB.tile([DK, NQ], f32, tag=f"pa{half}")
                for t in range(TCH):
                    nc.tensor.matmul(
                        pa, s_V[:, t, DK * h:DK * (h + 1)],
                        w_tiles[t][h // 4][:, NQ * (h % 4):NQ * (h % 4 + 1)],
                        start=(t == 0), stop=(t == TCH - 1))
                nc.vector.tensor_mul(
                    s_attn[h], pa,
                    rbq[h // 4][:, NQ * (h % 4):NQ * (h % 4 + 1)])

            po = psB.tile([NQ, E], f32, tag="pind")
            for h in range(H):
                nc.tensor.matmul(po, s_attn[h], s_wo[:, h, :],
                                 start=(h == 0), stop=(h == H - 1))
            s_out = big.tile([NQ, E], f32)
            nc.vector.tensor_add(s_out, po, s_bo2)
            nc.sync.dma_start(out=out[:, :], in_=s_out)

    nc.finalize()
    return nc


_NC_CACHE = {}


def _get_nc():
    if "nc" not in _NC_CACHE:
        _NC_CACHE["nc"] = build_nc()
    return _NC_CACHE["nc"]


def prep_inputs(x, Wq, bq_, Wk, bk_, Wv, bv_, Wo, bo_, iq_W, iq_b, ik_W, ik_b,
                wp_W, wp_b):
    bf = ml_dtypes.bfloat16
    f32 = np.float32
    xf = np.ascontiguousarray(np.asarray(x).reshape(S, E).astype(f32))
    xT = np.ascontiguousarray(xf.T)
    bd = np.zeros((128, 160), f32)
    for hl in range(4):
        for s_ in range(32):
            bd[32 * hl + s_, 64 + s_] = 1.0
    shared = {
        "xT": xT,
        "iqW": np.ascontiguousarray(iq_W, f32),
        "ikW": np.ascontiguousarray(ik_W, f32),
        "wpW": np.ascontiguousarray(wp_W, f32),
        "wq16": np.ascontiguousarray(Wq).astype(bf),
        "wk16": np.ascontiguousarray(Wk).astype(bf),
        "wv16": np.ascontiguousarray(Wv).astype(bf),
        "wo16": np.ascontiguousarray(
            np.asarray(Wo, f32).reshape(H, DK, E).transpose(1, 0, 2)).astype(bf),
        "iqb": np.ascontiguousarray(iq_b.reshape(E, 1), f32),
        "ikb": np.ascontiguousarray(ik_b.reshape(DK, 1), f32),
        "wpb": np.ascontiguousarray(wp_b.reshape(H, 1), f32),
        "bqh": np.ascontiguousarray(bq_.reshape(H, DK).T, f32),
        "bo2": np.ascontiguousarray(
            (np.asarray(bv_, np.float64) @ np.asarray(Wo, np.float64)
             + np.asarray(bo_, np.float64)).reshape(1, E)).astype(f32),
        "bd01": bd,
        "nramp": (-np.arange(S, dtype=np.float64) * RAMP_EPS
                  ).astype(f32).reshape(1, S),
        "ones96": np.ones((1, NQ), f32),
        "col16": np.arange(16, dtype=f32).reshape(1, 16),
    }
    in_maps = []
    for c in range(NC):
        m = dict(shared)
        xq = np.ascontiguousarray(xT[:, NQ * c:NQ * (c + 1)])
        m["xTq"] = xq
        in_maps.append(m)
    return in_maps


def kernel(**inputs):
    from concourse.bass_utils import run_bass_kernel_spmd
    nc = _get_nc()
    in_maps = prep_inputs(
        inputs["x"], inputs["Wq"], inputs["bq"], inputs["Wk"], inputs["bk"],
        inputs["Wv"], inputs["bv"], inputs["Wo"], inputs["bo"],
        inputs["iq_W"], inputs["iq_b"], inputs["ik_W"], inputs["ik_b"],
        inputs["wp_W"], inputs["wp_b"])
    res = run_bass_kernel_spmd(nc, in_maps, core_ids=list(range(NC)))
    outs = [res.results[c]["out"] for c in range(NC)]
    return np.concatenate(outs, axis=0)[None].astype(np.float32)



# revision 33
# speedup vs baseline: 2.0474x; 2.0474x over previous
"""DeepSeek sparse attention TRN2 kernel: 8-core query-parallel, v2.

Hardcoded for B=1, S=768, E=512, H=8, DK=64, TOPK=384, 8 cores.
  - Core c owns queries [96c, 96c+96). Output = host concat of per-core rows.
  - Indexer chain in fp32 (top-k set needs ~1e-6 score accuracy: a single
    flipped boundary key costs ~2e-2 output error).
  - Per-head score matmuls straight out of the projection PSUM slices;
    combine = relu*w (DVE STT) + running add-chain on GpSimd, seeded with
    the -t*2^-40 tie-break ramp via partition_broadcast.
  - Top-k via per-row threshold: 8 bisection steps (DVE is_ge counts),
    then exact top-16 fixup (max8 + match_replace + max8).
  - Attention = dense QK^T + multiplicative 0/1 mask, bf16. Softmax
    denominators via ones-row matmul; 1/den broadcast across partitions
    with gpsimd.partition_broadcast (no DRAM round trip).
  - bk dropped (softmax shift-invariance); bv folded into bo2 on host.
  - Inputs split across three DMA queues (SP / ACT / GpSimd) in k-chunks
    so PE starts ~1.3us in; all small constants ride in one packed DMA.
"""
import numpy as np
import ml_dtypes

S, E, H, DK = 768, 512, 8, 64
NQ = 96
NC = 8
KCH = 4            # 512/128 contraction chunks
TCH = 6            # 768/128 key chunks
TH = 384           # half of S; fp32 PSUM-bank-sized N
SCALING = 1.0 / np.sqrt(DK)
RAMP_EPS = float(2.0 ** -40)
R_ITERS = 8
BRK = 1.2
NEG = -1e30

# const pack column layout
C_IQB = 0          # [128, 4]
C_BQH = 4          # [128, 4]
C_IKB = 8          # [64, 1]
C_WPB = 9          # [1, 8]
C_RAMP = 17        # [1, 768]
C_COL16 = 785      # [96, 16]
C_BO2 = 801        # [96, 512]
C_ONES = 1313      # [1, 96]
CC = 1409


def build_nc(stage=99):
    import concourse.bass as bass
    import concourse.bacc as bacc
    from concourse import mybir
    from concourse.tile import TileContext

    f32 = mybir.dt.float32
    bf16 = mybir.dt.bfloat16
    u8 = mybir.dt.uint8
    AF = mybir.ActivationFunctionType
    OP = mybir.AluOpType

    nc = bacc.Bacc("TRN2", target_bir_lowering=False, debug=False)

    def din(name, shape, dt):
        return nc.dram_tensor(name, shape, dt, kind="ExternalInput")

    xT = din("xT", [E, S], f32)
    xTq = din("xTq", [E, NQ], f32)
    iqW = din("iqW", [E, E], f32)
    ikW = din("ikW", [E, 2 * DK], f32)
    wpW = din("wpW", [E, H], f32)
    xT16 = din("xT16", [E, S], bf16)
    wq16 = din("wq16", [E, E], bf16)
    wk16 = din("wk16", [E, E], bf16)
    wv16 = din("wv16", [E, E], bf16)
    wo2 = din("wo2", [128, KCH, E], bf16)
    constf = din("constf", [128, CC], f32)
    idb = din("idb", [NQ, NQ], bf16)
    out = nc.dram_tensor("out", [NQ, E], f32, kind="ExternalOutput")

    import contextlib
    with TileContext(nc) as tc:
      with contextlib.suppress(StopIteration):
        with tc.tile_pool(name="w1", bufs=1) as w1, \
             tc.tile_pool(name="big", bufs=1) as big, \
             tc.tile_pool(name="tiny", bufs=1) as tiny, \
             tc.tile_pool(name="psS", bufs=1, space="PSUM") as psS, \
             tc.tile_pool(name="psR", bufs=4, space="PSUM") as psR:

            # ---------------- SBUF tiles ----------------
            s_xT = w1.tile([128, KCH, S], f32)
            s_xTq = w1.tile([128, KCH, NQ], f32)
            s_xq16 = w1.tile([128, KCH, NQ], bf16)
            s_iqW = w1.tile([128, KCH, E], f32)
            s_ikW = w1.tile([128, KCH, 2 * DK], f32)
            s_wpW = w1.tile([128, KCH, H], f32)
            s_xT16 = w1.tile([128, KCH, S], bf16)
            s_wq = w1.tile([128, KCH, E], bf16)
            s_wk = w1.tile([128, KCH, E], bf16)
            s_wv = w1.tile([128, KCH, E], bf16)
            s_wo2 = w1.tile([128, KCH, E], bf16)
            s_const = w1.tile([128, CC], f32)
            s_idb = w1.tile([NQ, NQ], bf16)

            # ---------------- DMA: three queues ----------------
            # SP queue: xTq, xT chunks, then bf16 weights
            warm = tiny.tile([1, 1], f32)
            nc.vector.memset(warm, 0.0)
            warmR = tiny.tile([128, E], bf16, name="warmR")
            nc.vector.memset(warmR, 0.0)
            # SP queue: xTq, xT k0/k1, then bf16 weights
            nc.sync.dma_start(
                out=s_xTq, in_=xTq[:, :].rearrange("(k p) n -> p k n", p=128))
            for k in range(2):
                nc.sync.dma_start(
                    out=s_xT[:, k, :], in_=xT[128 * k:128 * (k + 1), :])
            nc.sync.dma_start(
                out=s_wk, in_=wk16[:, :].rearrange("(k p) n -> p k n", p=128))
            nc.sync.dma_start(
                out=s_xT16, in_=xT16[:, :].rearrange("(k p) n -> p k n", p=128))
            nc.sync.dma_start(
                out=s_wq, in_=wq16[:, :].rearrange("(k p) n -> p k n", p=128))
            nc.sync.dma_start(
                out=s_wv, in_=wv16[:, :].rearrange("(k p) n -> p k n", p=128))
            nc.sync.dma_start(out=s_wo2, in_=wo2[:, :, :])
            # ACT queue: iqW k0, table-load warmup, iqW k1-k3
            nc.scalar.dma_start(out=s_iqW[:, 0, :], in_=iqW[0:128, :])
            nc.scalar.activation(out=warm, in_=warm, func=AF.Identity,
                                 bias=0.0)
            for k in range(1, KCH):
                nc.scalar.dma_start(
                    out=s_iqW[:, k, :], in_=iqW[128 * k:128 * (k + 1), :])
            # Pool queue: ikW, xT k2/k3, wpW, const
            nc.gpsimd.dma_start(
                out=s_ikW, in_=ikW[:, :].rearrange("(k p) n -> p k n", p=128))
            for k in range(2, KCH):
                nc.gpsimd.dma_start(
                    out=s_xT[:, k, :], in_=xT[128 * k:128 * (k + 1), :])
            nc.gpsimd.dma_start(
                out=s_wpW, in_=wpW[:, :].rearrange("(k p) n -> p k n", p=128))
            nc.gpsimd.dma_start(out=s_const, in_=constf[:, :])
            nc.gpsimd.dma_start(out=s_idb, in_=idb[:, :])

            onesrow = tiny.tile([128, 1], bf16)
            nc.vector.memset(onesrow, 1.0)
            negbig = tiny.tile([NQ, 1], f32, name="negbig")
            nc.vector.memset(negbig, NEG)

            nc.vector.tensor_copy(s_xq16.rearrange("p k n -> p (k n)"),
                                  s_xTq.rearrange("p k n -> p (k n)"))

            # PE p-state warmup: cheap dummy matmuls keep the ramp anchored
            for i in range(5):
                pwm = psR.tile([H, E], f32, tag="ps", name="pwm")
                nc.tensor.matmul(pwm, warmR[:, 0:H], warmR,
                                 start=True, stop=True)

            # =========== INDEXER projections (fp32) ===========
            pq = [psR.tile([128, NQ], f32, tag="ps", name=f"pq{m}")
                  for m in range(KCH)]
            pk0 = psS.tile([128, TH], f32, tag="B")
            pk1 = psS.tile([128, TH], f32, tag="C")
            pw = psR.tile([NQ, H], f32, tag="ps", name="pw")
            for k in range(KCH):
                nc.tensor.matmul(pw, s_xTq[:, k, :], s_wpW[:, k, :],
                                 start=(k == 0), stop=False)
                for th, pk in ((0, pk0), (1, pk1)):
                    nc.tensor.matmul(pk, s_ikW[:, k, :],
                                     s_xT[:, k, TH * th:TH * (th + 1)],
                                     start=(k == 0), stop=(k == KCH - 1))
            for m in range(KCH):
                for k in range(KCH):
                    nc.tensor.matmul(pq[m],
                                     s_iqW[:, k, 128 * m:128 * (m + 1)],
                                     s_xTq[:, k, :],
                                     start=(k == 0), stop=(k == KCH - 1))
            # wTq bias as a K=1 matmul: ones96^T @ wpb
            nc.tensor.matmul(pw, s_const[0:1, C_ONES:C_ONES + NQ],
                             s_const[0:1, C_WPB:C_WPB + H],
                             start=False, stop=True)

            s_qid = big.tile([128, KCH, NQ], f32)
            s_kidT = big.tile([128, 2, TH], f32)
            s_wTq = tiny.tile([NQ, H], f32)
            for m in range(KCH):
                nc.scalar.activation(out=s_qid[:, m, :], in_=pq[m],
                                     func=AF.Identity,
                                     bias=s_const[:, C_IQB + m:C_IQB + m + 1])
            for th, pk in ((0, pk0), (1, pk1)):
                nc.scalar.activation(out=s_kidT[:, th, :], in_=pk,
                                     func=AF.Identity,
                                     bias=s_const[:, C_IKB:C_IKB + 1])
            nc.vector.tensor_copy(s_wTq, pw)

            # combine accumulator seeded with the tie-break ramp
            s_ind = big.tile([NQ, S], f32)
            nc.gpsimd.partition_broadcast(
                s_ind, s_const[0:1, C_RAMP:C_RAMP + S], channels=NQ)

            # =========== per-head scores + combine ===========
            ws = [big.tile([NQ, 2, TH], f32, tag=f"ws{h}", name=f"ws{h}")
                  for h in range(H)]
            for h in range(H):
                p0 = DK * (h % 2)
                lhs = s_qid[p0:p0 + DK, h // 2, :]
                for th in range(2):
                    psc = psR.tile([NQ, TH], f32, tag="ps")
                    nc.tensor.matmul(psc, lhs, s_kidT[p0:p0 + DK, th, :],
                                     start=True, stop=True)
                    nc.vector.scalar_tensor_tensor(
                        out=ws[h][:, th, :], in0=psc, scalar=0.0,
                        in1=s_wTq[:, h:h + 1].to_broadcast([NQ, TH]),
                        op0=OP.max, op1=OP.mult)
                nc.gpsimd.tensor_add(
                    s_ind.rearrange("p n -> p n"),
                    s_ind.rearrange("p n -> p n"),
                    ws[h].rearrange("p a b -> p (a b)"))

            if stage == 1:
                s_oY = big.tile([NQ, E], f32, name="s_oY")
                nc.vector.memset(s_oY, 0.0)
                nc.vector.tensor_copy(s_oY[:, 0:2], s_ind[:, 0:2])
                nc.sync.dma_start(out=out[:, :], in_=s_oY)
                raise StopIteration
            # =========== TOPK threshold (bisection on DVE) ===========
            hp = tc.high_priority()
            hp.__enter__()
            rsum = tiny.tile([NQ, 1], f32, name="rsum")
            scr = big.tile([NQ, S], bf16, tag="scr")
            nc.vector.tensor_scalar(scr, s_ind, 0.0, None, op0=OP.add,
                                    op1=OP.add, accum_out=rsum)
            lo = tiny.tile([NQ, 1], f32)
            hi = tiny.tile([NQ, 1], f32)
            tmp = tiny.tile([NQ, 1], f32)
            mid = tiny.tile([NQ, 1], f32)
            cnt = tiny.tile([NQ, 1], f32)
            cmp = tiny.tile([NQ, 1], u8)
            ncmp = tiny.tile([NQ, 1], u8)
            nc.vector.tensor_scalar(lo, rsum, 1.0 / S, -BRK, op0=OP.mult,
                                    op1=OP.add)
            nc.vector.tensor_scalar(hi, rsum, 1.0 / S, BRK, op0=OP.mult,
                                    op1=OP.add)
            for r in range(R_ITERS):
                nc.vector.tensor_add(tmp, lo, hi)
                nc.vector.tensor_scalar_mul(mid, tmp, 0.5)
                nc.vector.tensor_scalar(scr, s_ind, mid, None, op0=OP.is_ge,
                                        op1=OP.add, accum_out=cnt)
                nc.vector.tensor_scalar(cmp, cnt, 384.0, None, op0=OP.is_ge)
                nc.vector.tensor_scalar(ncmp, cnt, 384.0, None, op0=OP.is_lt)
                nc.vector.copy_predicated(lo, cmp, mid)
                nc.vector.copy_predicated(hi, ncmp, mid)

            # exact count at hi; in-bracket top-16
            c_hi = tiny.tile([NQ, 1], f32)
            nc.vector.tensor_scalar(scr, s_ind, hi, None, op0=OP.is_ge,
                                    op1=OP.add, accum_out=c_hi)
            hicut = big.tile([NQ, S], f32, tag="hicut")
            nc.vector.scalar_tensor_tensor(
                out=hicut, in0=s_ind, scalar=hi,
                in1=negbig.to_broadcast([NQ, S]), op0=OP.is_ge, op1=OP.mult)
            mlo = big.tile([NQ, S], f32, tag="mlo")
            nc.vector.tensor_add(mlo[:, :TH], hicut[:, :TH], s_ind[:, :TH])
            nc.gpsimd.tensor_add(mlo[:, TH:], hicut[:, TH:], s_ind[:, TH:])
            m16 = tiny.tile([NQ, 16], f32)
            mlo2 = big.tile([NQ, S], f32, tag="mlo2")
            nc.vector.max(out=m16[:, 0:8], in_=mlo)
            nc.vector.match_replace(out=mlo2, in_to_replace=m16[:, 0:8],
                                    in_values=mlo, imm_value=NEG)
            nc.vector.max(out=m16[:, 8:16], in_=mlo2)
            need_m1 = tiny.tile([NQ, 1], f32)
            nc.vector.tensor_scalar(need_m1, c_hi, -1.0, 383.0, op0=OP.mult,
                                    op1=OP.add)
            oh = tiny.tile([NQ, 16], f32)
            tstar = tiny.tile([NQ, 1], f32)
            nc.vector.tensor_scalar(oh, s_const[:NQ, C_COL16:C_COL16 + 16],
                                    need_m1, None, op0=OP.is_equal)
            nc.vector.scalar_tensor_tensor(out=oh, in0=m16, scalar=1.0,
                                           in1=oh, op0=OP.mult, op1=OP.mult,
                                           accum_out=tstar)
            mask01 = big.tile([NQ, S], bf16, tag="mask01")
            nc.vector.tensor_scalar(mask01, s_ind, tstar, None, op0=OP.is_ge)
            s_maskT = big.tile([128, TCH, NQ], bf16)
            pmT = psS.tile([128, TCH, NQ], bf16, tag="D", name="pmT")
            for t in range(TCH):
                nc.tensor.transpose(pmT[:, t, :],
                                    mask01[:, 128 * t:128 * (t + 1)], s_idb)
            nc.vector.tensor_copy(
                s_maskT.rearrange("p a b -> p (a b)"),
                pmT.rearrange("p a b -> p (a b)"))
            gate = tiny.tile([128, 1], bf16, name="gate")
            nc.vector.tensor_scalar(gate, s_maskT[:, 0, 0:1], 0.0, 1.0,
                                    op0=OP.mult, op1=OP.add)
            hp.__exit__(None, None, None)
            if stage == 2:
                s_oX = big.tile([NQ, E], f32, name="s_oX")
                nc.vector.memset(s_oX, 0.0)
                nc.vector.tensor_copy(s_oX[:, 0:1], s_maskT[:96, 0, 0:1])
                nc.sync.dma_start(out=out[:, :], in_=s_oX)
                raise StopIteration

            # =========== ATTENTION (bf16) ===========
            s_KT = big.tile([128, KCH, S], bf16)
            s_QT = big.tile([128, KCH, NQ], bf16)
            s_V = big.tile([128, TCH, E], bf16)
            for m in range(KCH):
                for th in range(2):
                    pk2 = psR.tile([128, TH], f32, tag="ps")
                    for k in range(KCH):
                        nc.tensor.matmul(pk2,
                                         s_wk[:, k, 128 * m:128 * (m + 1)],
                                         s_xT16[:, k, TH * th:TH * (th + 1)],
                                         start=(k == 0), stop=(k == KCH - 1))
                    nc.scalar.copy(s_KT[:, m, TH * th:TH * (th + 1)], pk2)
            for m in range(KCH):
                pq2 = psR.tile([128, NQ], f32, tag="ps")
                for k in range(KCH):
                    nc.tensor.matmul(pq2, s_wq[:, k, 128 * m:128 * (m + 1)],
                                     s_xq16[:, k, :],
                                     start=(k == 0), stop=(k == KCH - 1))
                nc.scalar.activation(out=s_QT[:, m, :], in_=pq2,
                                     func=AF.Identity,
                                     bias=s_const[:, C_BQH + m:C_BQH + m + 1])

            if stage == 25:
                s_oW = big.tile([NQ, E], f32, name="s_oW")
                nc.vector.memset(s_oW, 0.0)
                nc.vector.tensor_copy(s_oW[:, 0:1], s_KT[:96, 0, 0:1])
                nc.vector.tensor_copy(s_oW[:, 1:2], s_QT[:96, 0, 0:1])
                nc.sync.dma_start(out=out[:, :], in_=s_oW)
                raise StopIteration
            # QK^T -> exp (q0 tiles first so exp starts as soon as KT m0/m1
            # land), V projection MMs interleaved to fill the exp-paced gaps
            w_tiles = [[big.tile([128, 4 * NQ], bf16, tag=f"wt_{t}_{q}",
                                 name=f"wt_{t}_{q}") for q in range(2)]
                       for t in range(TCH)]
            def emit_qk(t, q):
                psc2 = psR.tile([128, 4 * NQ], f32, tag="ps", name="psc2")
                p0 = DK * q
                for j in range(4):
                    nc.tensor.matmul(
                        psc2[:, NQ * j:NQ * (j + 1)],
                        s_KT[p0:p0 + DK, j, 128 * t:128 * (t + 1)],
                        s_QT[p0:p0 + DK, j, :],
                        start=True, stop=True)
                nc.scalar.activation(out=w_tiles[t][q], in_=psc2,
                                     func=AF.Exp, scale=SCALING)
            def emit_v(t):
                pv = psR.tile([128, E], f32, tag="ps", name="pv")
                for k in range(KCH):
                    nc.tensor.matmul(pv, s_xT16[:, k, 128 * t:128 * (t + 1)],
                                     s_wv[:, k, :],
                                     start=(k == 0), stop=(k == KCH - 1))
                nc.scalar.copy(s_V[:, t, :], pv)
            for t in range(TCH):
                emit_qk(t, 0)
                if stage != 27:
                    emit_v(t)
            for t in range(TCH):
                emit_qk(t, 1)

            if stage in (26, 27):
                s_oV = big.tile([NQ, E], f32, name="s_oV")
                nc.vector.memset(s_oV, 0.0)
                nc.vector.tensor_copy(s_oV[:, 0:1], w_tiles[0][0][:96, 0:1])
                nc.vector.tensor_copy(s_oV[:, 1:2], w_tiles[5][1][:96, 0:1])
                nc.sync.dma_start(out=out[:, :], in_=s_oV)
                raise StopIteration
            if stage == 3:
                s_oZ = big.tile([NQ, E], f32, name="s_oZ")
                nc.vector.memset(s_oZ, 0.0)
                nc.vector.tensor_copy(s_oZ[:, 0:1], s_V[:96, 0, 0:1])
                nc.vector.tensor_copy(s_oZ[:, 1:2], w_tiles[5][1][:96, 0:1])
                nc.sync.dma_start(out=out[:, :], in_=s_oZ)
                raise StopIteration
            # mask -> den -> AV -> out
            pden = [psS.tile([1, 4 * NQ], f32, tag=t_, name=f"pden{t_}")
                    for t_ in ("C", "D")]
            pa = [psS.tile([DK, 4, NQ], f32, tag=t_, name=f"pa{t_}")
                  for t_ in ("A", "B")]
            for ti, t in enumerate((0, 1, 2, 3, 4, 5)):
                msl = s_maskT[:, t, :]
                mrep = bass.AP(tensor=msl.tensor, offset=msl.offset,
                               ap=[msl.ap[0], [0, 4]] + msl.ap[1:])
                for q in range(2):
                    wt = w_tiles[t][q]
                    eng = nc.vector if q == 0 else nc.gpsimd
                    eng.tensor_mul(wt, wt, mrep)
                    nc.tensor.matmul(pden[q], onesrow, wt,
                                     start=(ti == 0), stop=(ti == TCH - 1))
            for ti, t in enumerate((0, 1, 2, 3, 4, 5)):
                for q in range(2):
                    for j in range(4):
                        h = 2 * j + q
                        nc.tensor.matmul(
                            pa[q][:, j, :],
                            s_V[:, t, DK * h:DK * (h + 1)],
                            w_tiles[t][q][:, NQ * j:NQ * (j + 1)],
                            start=(ti == 0 and j == 0 and True),
                            stop=(ti == TCH - 1 and j == 3))
            s_rden = tiny.tile([1, 2, 4 * NQ], f32)
            s_rdenb = big.tile([DK, 2, 4 * NQ], f32)
            for q in range(2):
                nc.vector.reciprocal(s_rden[:, q, :], pden[q])
                nc.gpsimd.partition_broadcast(
                    s_rdenb[:, q, :], s_rden[0:1, q, :], channels=DK)
            s_attn2 = big.tile([128, KCH, NQ], bf16)
            for q in range(2):
                nc.vector.tensor_mul(
                    s_attn2[DK * q:DK * q + DK, :, :].rearrange(
                        "p a b -> p (a b)"),
                    pa[q][:, :, :].rearrange("p a b -> p (a b)"),
                    s_rdenb[:, q, :])
            po_a = psS.tile([NQ, E // 2], f32, tag="D", name="po_a")
            po_b = psS.tile([NQ, E // 2], f32, tag="C", name="po_b")
            s_out = big.tile([NQ, E], f32)
            for half, po in ((0, po_a), (1, po_b)):
                e0 = half * (E // 2)
                for j in range(KCH):
                    nc.tensor.matmul(po, s_attn2[:, j, :],
                                     s_wo2[:, j, e0:e0 + E // 2],
                                     start=(j == 0), stop=(j == KCH - 1))
                nc.vector.tensor_add(
                    s_out[:, e0:e0 + E // 2], po,
                    s_const[:NQ, C_BO2 + e0:C_BO2 + e0 + E // 2])
                nc.sync.dma_start(out=out[:, e0:e0 + E // 2],
                                  in_=s_out[:, e0:e0 + E // 2])

    nc.finalize()
    return nc


_NC_CACHE = {}


def _get_nc():
    if "nc" not in _NC_CACHE:
        _NC_CACHE["nc"] = build_nc()
    return _NC_CACHE["nc"]


def prep_inputs(x, Wq, bq_, Wk, bk_, Wv, bv_, Wo, bo_, iq_W, iq_b, ik_W, ik_b,
                wp_W, wp_b):
    bf = ml_dtypes.bfloat16
    f32 = np.float32
    xf = np.ascontiguousarray(np.asarray(x).reshape(S, E).astype(f32))
    xT = np.ascontiguousarray(xf.T)

    const = np.zeros((128, CC), f32)
    const[:, C_IQB:C_IQB + 4] = np.asarray(iq_b, f32).reshape(4, 128).T
    bqh = np.asarray(bq_, np.float64).reshape(H, DK)  # [head, d]
    for m in range(KCH):
        for a in range(2):
            const[64 * a:64 * a + 64, C_BQH + m] = bqh[2 * m + a]
    const[:DK, C_IKB] = np.asarray(ik_b, f32)
    const[DK:128, C_IKB] = np.asarray(ik_b, f32)
    const[0, C_WPB:C_WPB + H] = np.asarray(wp_b, f32)
    const[0, C_RAMP:C_RAMP + S] = (-np.arange(S, dtype=np.float64)
                                   * RAMP_EPS).astype(f32)
    const[:NQ, C_COL16:C_COL16 + 16] = np.arange(16, dtype=f32)
    bo2 = (np.asarray(bv_, np.float64) @ np.asarray(Wo, np.float64)
           + np.asarray(bo_, np.float64)).astype(f32)
    const[:NQ, C_BO2:C_BO2 + E] = bo2
    const[0, C_ONES:C_ONES + NQ] = 1.0

    W3 = np.asarray(Wo, f32).reshape(H, DK, E)
    wo2 = np.zeros((128, KCH, E), f32)
    for j in range(KCH):
        for a in range(2):
            wo2[64 * a:64 * a + 64, j, :] = W3[2 * j + a]

    shared = {
        "xT": xT,
        "iqW": np.ascontiguousarray(iq_W, f32),
        "ikW": np.ascontiguousarray(
            np.concatenate([ik_W, ik_W], axis=1), f32),
        "wpW": np.ascontiguousarray(wp_W, f32),
        "xT16": np.ascontiguousarray(xT).astype(bf),
        "wq16": np.ascontiguousarray(Wq).astype(bf),
        "wk16": np.ascontiguousarray(Wk).astype(bf),
        "wv16": np.ascontiguousarray(Wv).astype(bf),
        "wo2": np.ascontiguousarray(wo2).astype(bf),
        "constf": const,
    }
    shared["idb"] = np.eye(NQ, dtype=np.float32).astype(bf)
    in_maps = []
    for c in range(NC):
        m = dict(shared)
        m["xTq"] = np.ascontiguousarray(xT[:, NQ * c:NQ * (c + 1)])
        in_maps.append(m)
    return in_maps


def kernel(**inputs):
    from concourse.bass_utils import run_bass_kernel_spmd
    nc = _get_nc()
    in_maps = prep_inputs(
        inputs["x"], inputs["Wq"], inputs["bq"], inputs["Wk"], inputs["bk"],
        inputs["Wv"], inputs["bv"], inputs["Wo"], inputs["bo"],
        inputs["iq_W"], inputs["iq_b"], inputs["ik_W"], inputs["ik_b"],
        inputs["wp_W"], inputs["wp_b"])
    res = run_bass_kernel_spmd(nc, in_maps, core_ids=list(range(NC)))
    outs = [res.results[c]["out"] for c in range(NC)]
    return np.concatenate(outs, axis=0)[None].astype(np.float32)


# revision 34
# speedup vs baseline: 2.0478x; 1.0002x over previous
"""DeepSeek sparse attention TRN2 kernel: 8-core query-parallel, v2.

Hardcoded for B=1, S=768, E=512, H=8, DK=64, TOPK=384, 8 cores.
  - Core c owns queries [96c, 96c+96). Output = host concat of per-core rows.
  - Indexer chain in fp32 (top-k set needs ~1e-6 score accuracy: a single
    flipped boundary key costs ~2e-2 output error).
  - Per-head score matmuls straight out of the projection PSUM slices;
    combine = relu*w (DVE STT) + running add-chain on GpSimd, seeded with
    the -t*2^-40 tie-break ramp via partition_broadcast.
  - Top-k via per-row threshold: 8 bisection steps (DVE is_ge counts),
    then exact top-16 fixup (max8 + match_replace + max8).
  - Attention = dense QK^T + multiplicative 0/1 mask, bf16. Softmax
    denominators via ones-row matmul; 1/den broadcast across partitions
    with gpsimd.partition_broadcast (no DRAM round trip).
  - bk dropped (softmax shift-invariance); bv folded into bo2 on host.
  - Inputs split across three DMA queues (SP / ACT / GpSimd) in k-chunks
    so PE starts ~1.3us in; all small constants ride in one packed DMA.
"""
import numpy as np
import ml_dtypes

S, E, H, DK = 768, 512, 8, 64
NQ = 96
NC = 8
KCH = 4            # 512/128 contraction chunks
TCH = 6            # 768/128 key chunks
TH = 384           # half of S; fp32 PSUM-bank-sized N
SCALING = 1.0 / np.sqrt(DK)
RAMP_EPS = float(2.0 ** -40)
R_ITERS = 8
BRK = 1.2
NEG = -1e30

# const pack column layout
C_IQB = 0          # [128, 4]
C_BQH = 4          # [128, 4]
C_IKB = 8          # [64, 1]
C_WPB = 9          # [1, 8]
C_RAMP = 17        # [1, 768]
C_COL16 = 785      # [96, 16]
C_BO2 = 801        # [96, 512]
C_ONES = 1313      # [1, 96]
CC = 1409


def build_nc(stage=99):
    import concourse.bass as bass
    import concourse.bacc as bacc
    from concourse import mybir
    from concourse.tile import TileContext

    f32 = mybir.dt.float32
    bf16 = mybir.dt.bfloat16
    u8 = mybir.dt.uint8
    AF = mybir.ActivationFunctionType
    OP = mybir.AluOpType

    nc = bacc.Bacc("TRN2", target_bir_lowering=False, debug=False)

    def din(name, shape, dt):
        return nc.dram_tensor(name, shape, dt, kind="ExternalInput")

    xT = din("xT", [E, S], f32)
    xTq = din("xTq", [E, NQ], f32)
    iqW = din("iqW", [E, E], f32)
    ikW = din("ikW", [E, 2 * DK], f32)
    wpW = din("wpW", [E, H], f32)
    xT16 = din("xT16", [E, S], bf16)
    wq16 = din("wq16", [E, E], bf16)
    wk16 = din("wk16", [E, E], bf16)
    wv16 = din("wv16", [E, E], bf16)
    wo2 = din("wo2", [128, KCH, E], bf16)
    constf = din("constf", [128, CC], f32)
    idb = din("idb", [NQ, NQ], bf16)
    out = nc.dram_tensor("out", [NQ, E], f32, kind="ExternalOutput")

    import contextlib
    with TileContext(nc) as tc:
      with contextlib.suppress(StopIteration):
        with tc.tile_pool(name="w1", bufs=1) as w1, \
             tc.tile_pool(name="big", bufs=1) as big, \
             tc.tile_pool(name="tiny", bufs=1) as tiny, \
             tc.tile_pool(name="psS", bufs=1, space="PSUM") as psS, \
             tc.tile_pool(name="psR", bufs=4, space="PSUM") as psR:

            # ---------------- SBUF tiles ----------------
            s_xT = w1.tile([128, KCH, S], f32)
            s_xTq = w1.tile([128, KCH, NQ], f32)
            s_xq16 = w1.tile([128, KCH, NQ], bf16)
            s_iqW = w1.tile([128, KCH, E], f32)
            s_ikW = w1.tile([128, KCH, 2 * DK], f32)
            s_wpW = w1.tile([128, KCH, H], f32)
            s_xT16 = w1.tile([128, KCH, S], bf16)
            s_wq = w1.tile([128, KCH, E], bf16)
            s_wk = w1.tile([128, KCH, E], bf16)
            s_wv = w1.tile([128, KCH, E], bf16)
            s_wo2 = w1.tile([128, KCH, E], bf16)
            s_const = w1.tile([128, CC], f32)
            s_idb = w1.tile([NQ, NQ], bf16)

            # ---------------- DMA: three queues ----------------
            # SP queue: xTq, xT chunks, then bf16 weights
            warm = tiny.tile([1, 1], f32)
            nc.vector.memset(warm, 0.0)
            warmR = tiny.tile([128, E], bf16, name="warmR")
            nc.vector.memset(warmR, 0.0)
            # SP queue: xTq, xT k0/k1, then bf16 weights
            nc.sync.dma_start(
                out=s_xTq, in_=xTq[:, :].rearrange("(k p) n -> p k n", p=128))
            for k in range(2):
                nc.sync.dma_start(
                    out=s_xT[:, k, :], in_=xT[128 * k:128 * (k + 1), :])
            nc.sync.dma_start(
                out=s_wk, in_=wk16[:, :].rearrange("(k p) n -> p k n", p=128))
            nc.sync.dma_start(
                out=s_xT16, in_=xT16[:, :].rearrange("(k p) n -> p k n", p=128))
            nc.sync.dma_start(
                out=s_wq, in_=wq16[:, :].rearrange("(k p) n -> p k n", p=128))
            nc.sync.dma_start(
                out=s_wv, in_=wv16[:, :].rearrange("(k p) n -> p k n", p=128))
            nc.sync.dma_start(out=s_wo2, in_=wo2[:, :, :])
            # ACT queue: iqW k0, table-load warmup, iqW k1-k3
            nc.scalar.dma_start(out=s_iqW[:, 0, :], in_=iqW[0:128, :])
            nc.scalar.activation(out=warm, in_=warm, func=AF.Identity,
                                 bias=0.0)
            for k in range(1, KCH):
                nc.scalar.dma_start(
                    out=s_iqW[:, k, :], in_=iqW[128 * k:128 * (k + 1), :])
            # Pool queue: ikW, xT k2/k3, wpW, const
            nc.gpsimd.dma_start(
                out=s_ikW, in_=ikW[:, :].rearrange("(k p) n -> p k n", p=128))
            for k in range(2, KCH):
                nc.gpsimd.dma_start(
                    out=s_xT[:, k, :], in_=xT[128 * k:128 * (k + 1), :])
            nc.gpsimd.dma_start(
                out=s_wpW, in_=wpW[:, :].rearrange("(k p) n -> p k n", p=128))
            nc.gpsimd.dma_start(out=s_const, in_=constf[:, :])
            nc.gpsimd.dma_start(out=s_idb, in_=idb[:, :])

            onesrow = tiny.tile([128, 1], bf16)
            nc.vector.memset(onesrow, 1.0)
            negbig = tiny.tile([NQ, 1], f32, name="negbig")
            nc.vector.memset(negbig, NEG)

            nc.vector.tensor_copy(s_xq16.rearrange("p k n -> p (k n)"),
                                  s_xTq.rearrange("p k n -> p (k n)"))

            # PE p-state warmup: cheap dummy matmuls keep the ramp anchored
            for i in range(5):
                pwm = psR.tile([H, E], f32, tag="ps", name="pwm")
                nc.tensor.matmul(pwm, warmR[:, 0:H], warmR,
                                 start=True, stop=True)

            # =========== INDEXER projections (fp32) ===========
            pq = [psR.tile([128, NQ], f32, tag="ps", name=f"pq{m}")
                  for m in range(KCH)]
            pk0 = psS.tile([128, TH], f32, tag="B")
            pk1 = psS.tile([128, TH], f32, tag="C")
            pw = psR.tile([NQ, H], f32, tag="ps", name="pw")
            for k in range(KCH):
                nc.tensor.matmul(pw, s_xTq[:, k, :], s_wpW[:, k, :],
                                 start=(k == 0), stop=False)
                for th, pk in ((0, pk0), (1, pk1)):
                    nc.tensor.matmul(pk, s_ikW[:, k, :],
                                     s_xT[:, k, TH * th:TH * (th + 1)],
                                     start=(k == 0), stop=(k == KCH - 1))
            for m in range(KCH):
                for k in range(KCH):
                    nc.tensor.matmul(pq[m],
                                     s_iqW[:, k, 128 * m:128 * (m + 1)],
                                     s_xTq[:, k, :],
                                     start=(k == 0), stop=(k == KCH - 1))
            # wTq bias as a K=1 matmul: ones96^T @ wpb
            nc.tensor.matmul(pw, s_const[0:1, C_ONES:C_ONES + NQ],
                             s_const[0:1, C_WPB:C_WPB + H],
                             start=False, stop=True)

            s_qid = big.tile([128, KCH, NQ], f32)
            s_kidT = big.tile([128, 2, TH], f32)
            s_wTq = tiny.tile([NQ, H], f32)
            for m in range(KCH):
                nc.scalar.activation(out=s_qid[:, m, :], in_=pq[m],
                                     func=AF.Identity,
                                     bias=s_const[:, C_IQB + m:C_IQB + m + 1])
            for th, pk in ((0, pk0), (1, pk1)):
                nc.scalar.activation(out=s_kidT[:, th, :], in_=pk,
                                     func=AF.Identity,
                                     bias=s_const[:, C_IKB:C_IKB + 1])
            nc.vector.tensor_copy(s_wTq, pw)

            # combine accumulator seeded with the tie-break ramp
            s_ind = big.tile([NQ, S], f32)
            nc.gpsimd.partition_broadcast(
                s_ind, s_const[0:1, C_RAMP:C_RAMP + S], channels=NQ)

            # =========== per-head scores + combine ===========
            ws = [big.tile([NQ, 2, TH], f32, tag=f"ws{h}", name=f"ws{h}")
                  for h in range(H)]
            for h in range(H):
                p0 = DK * (h % 2)
                lhs = s_qid[p0:p0 + DK, h // 2, :]
                for th in range(2):
                    psc = psR.tile([NQ, TH], f32, tag="ps")
                    nc.tensor.matmul(psc, lhs, s_kidT[p0:p0 + DK, th, :],
                                     start=True, stop=True)
                    nc.vector.scalar_tensor_tensor(
                        out=ws[h][:, th, :], in0=psc, scalar=0.0,
                        in1=s_wTq[:, h:h + 1].to_broadcast([NQ, TH]),
                        op0=OP.max, op1=OP.mult)
                nc.gpsimd.tensor_add(
                    s_ind.rearrange("p n -> p n"),
                    s_ind.rearrange("p n -> p n"),
                    ws[h].rearrange("p a b -> p (a b)"))

            if stage == 1:
                s_oY = big.tile([NQ, E], f32, name="s_oY")
                nc.vector.memset(s_oY, 0.0)
                nc.vector.tensor_copy(s_oY[:, 0:2], s_ind[:, 0:2])
                nc.sync.dma_start(out=out[:, :], in_=s_oY)
                raise StopIteration
            # =========== TOPK threshold (bisection on DVE) ===========
            hp = tc.high_priority()
            hp.__enter__()
            rsum = tiny.tile([NQ, 1], f32, name="rsum")
            scr = big.tile([NQ, S], bf16, tag="scr")
            nc.vector.tensor_scalar(scr, s_ind, 0.0, None, op0=OP.add,
                                    op1=OP.add, accum_out=rsum)
            lo = tiny.tile([NQ, 1], f32)
            hi = tiny.tile([NQ, 1], f32)
            tmp = tiny.tile([NQ, 1], f32)
            mid = tiny.tile([NQ, 1], f32)
            cnt = tiny.tile([NQ, 1], f32)
            cmp = tiny.tile([NQ, 1], u8)
            ncmp = tiny.tile([NQ, 1], u8)
            nc.vector.tensor_scalar(lo, rsum, 1.0 / S, -BRK, op0=OP.mult,
                                    op1=OP.add)
            nc.vector.tensor_scalar(hi, rsum, 1.0 / S, BRK, op0=OP.mult,
                                    op1=OP.add)
            for r in range(R_ITERS):
                nc.vector.tensor_add(tmp, lo, hi)
                nc.vector.tensor_scalar_mul(mid, tmp, 0.5)
                nc.vector.tensor_scalar(scr, s_ind, mid, None, op0=OP.is_ge,
                                        op1=OP.add, accum_out=cnt)
                nc.vector.tensor_scalar(cmp, cnt, 384.0, None, op0=OP.is_ge)
                nc.vector.tensor_scalar(ncmp, cnt, 384.0, None, op0=OP.is_lt)
                nc.vector.copy_predicated(lo, cmp, mid)
                nc.vector.copy_predicated(hi, ncmp, mid)

            # exact count at hi; in-bracket top-16
            c_hi = tiny.tile([NQ, 1], f32)
            nc.vector.tensor_scalar(scr, s_ind, hi, None, op0=OP.is_ge,
                                    op1=OP.add, accum_out=c_hi)
            hicut = big.tile([NQ, S], f32, tag="hicut")
            nc.vector.scalar_tensor_tensor(
                out=hicut, in0=s_ind, scalar=hi,
                in1=negbig.to_broadcast([NQ, S]), op0=OP.is_ge, op1=OP.mult)
            mlo = big.tile([NQ, S], f32, tag="mlo")
            nc.vector.tensor_add(mlo[:, :TH], hicut[:, :TH], s_ind[:, :TH])
            nc.gpsimd.tensor_add(mlo[:, TH:], hicut[:, TH:], s_ind[:, TH:])
            m16 = tiny.tile([NQ, 16], f32)
            mlo2 = big.tile([NQ, S], f32, tag="mlo2")
            nc.vector.max(out=m16[:, 0:8], in_=mlo)
            nc.vector.match_replace(out=mlo2, in_to_replace=m16[:, 0:8],
                                    in_values=mlo, imm_value=NEG)
            nc.vector.max(out=m16[:, 8:16], in_=mlo2)
            need_m1 = tiny.tile([NQ, 1], f32)
            nc.vector.tensor_scalar(need_m1, c_hi, -1.0, 383.0, op0=OP.mult,
                                    op1=OP.add)
            oh = tiny.tile([NQ, 16], f32)
            tstar = tiny.tile([NQ, 1], f32)
            nc.vector.tensor_scalar(oh, s_const[:NQ, C_COL16:C_COL16 + 16],
                                    need_m1, None, op0=OP.is_equal)
            nc.vector.scalar_tensor_tensor(out=oh, in0=m16, scalar=1.0,
                                           in1=oh, op0=OP.mult, op1=OP.mult,
                                           accum_out=tstar)
            mask01 = big.tile([NQ, S], bf16, tag="mask01")
            nc.vector.tensor_scalar(mask01, s_ind, tstar, None, op0=OP.is_ge)
            s_maskT = big.tile([128, TCH, NQ], bf16)
            pmT = psS.tile([128, TCH, NQ], bf16, tag="D", name="pmT")
            for t in range(TCH):
                nc.tensor.transpose(pmT[:, t, :],
                                    mask01[:, 128 * t:128 * (t + 1)], s_idb)
            nc.vector.tensor_copy(
                s_maskT.rearrange("p a b -> p (a b)"),
                pmT.rearrange("p a b -> p (a b)"))
            gate = tiny.tile([128, 1], bf16, name="gate")
            nc.vector.tensor_scalar(gate, s_maskT[:, 0, 0:1], 0.0, 1.0,
                                    op0=OP.mult, op1=OP.add)
            hp.__exit__(None, None, None)
            if stage == 2:
                s_oX = big.tile([NQ, E], f32, name="s_oX")
                nc.vector.memset(s_oX, 0.0)
                nc.vector.tensor_copy(s_oX[:, 0:1], s_maskT[:96, 0, 0:1])
                nc.sync.dma_start(out=out[:, :], in_=s_oX)
                raise StopIteration

            # =========== ATTENTION (bf16) ===========
            s_KT = big.tile([128, KCH, S], bf16)
            s_QT = big.tile([128, KCH, NQ], bf16)
            s_V = big.tile([128, TCH, E], bf16)
            for m in range(KCH):
                for th in range(2):
                    pk2 = psR.tile([128, TH], f32, tag="ps")
                    for k in range(KCH):
                        nc.tensor.matmul(pk2,
                                         s_wk[:, k, 128 * m:128 * (m + 1)],
                                         s_xT16[:, k, TH * th:TH * (th + 1)],
                                         start=(k == 0), stop=(k == KCH - 1))
                    nc.scalar.copy(s_KT[:, m, TH * th:TH * (th + 1)], pk2)
            for m in range(KCH):
                pq2 = psR.tile([128, NQ], f32, tag="ps")
                for k in range(KCH):
                    nc.tensor.matmul(pq2, s_wq[:, k, 128 * m:128 * (m + 1)],
                                     s_xq16[:, k, :],
                                     start=(k == 0), stop=(k == KCH - 1))
                nc.scalar.activation(out=s_QT[:, m, :], in_=pq2,
                                     func=AF.Identity,
                                     bias=s_const[:, C_BQH + m:C_BQH + m + 1])

            if stage == 25:
                s_oW = big.tile([NQ, E], f32, name="s_oW")
                nc.vector.memset(s_oW, 0.0)
                nc.vector.tensor_copy(s_oW[:, 0:1], s_KT[:96, 0, 0:1])
                nc.vector.tensor_copy(s_oW[:, 1:2], s_QT[:96, 0, 0:1])
                nc.sync.dma_start(out=out[:, :], in_=s_oW)
                raise StopIteration
            # QK^T -> exp (q0 tiles first so exp starts as soon as KT m0/m1
            # land), V projection MMs interleaved to fill the exp-paced gaps
            w_tiles = [[big.tile([128, 4 * NQ], bf16, tag=f"wt_{t}_{q}",
                                 name=f"wt_{t}_{q}") for q in range(2)]
                       for t in range(TCH)]
            def emit_qk(t, q):
                psc2 = psR.tile([128, 4 * NQ], f32, tag="ps", name="psc2")
                p0 = DK * q
                for j in range(4):
                    nc.tensor.matmul(
                        psc2[:, NQ * j:NQ * (j + 1)],
                        s_KT[p0:p0 + DK, j, 128 * t:128 * (t + 1)],
                        s_QT[p0:p0 + DK, j, :],
                        start=True, stop=True)
                nc.scalar.activation(out=w_tiles[t][q], in_=psc2,
                                     func=AF.Exp, scale=SCALING)
            def emit_v(t):
                pv = psR.tile([128, E], f32, tag="ps", name="pv")
                for k in range(KCH):
                    nc.tensor.matmul(pv, s_xT16[:, k, 128 * t:128 * (t + 1)],
                                     s_wv[:, k, :],
                                     start=(k == 0), stop=(k == KCH - 1))
                nc.scalar.copy(s_V[:, t, :], pv)
            for t in range(TCH):
                emit_qk(t, 0)
                if stage != 27:
                    emit_v(t)
            for t in range(TCH):
                emit_qk(t, 1)

            if stage in (26, 27):
                s_oV = big.tile([NQ, E], f32, name="s_oV")
                nc.vector.memset(s_oV, 0.0)
                nc.vector.tensor_copy(s_oV[:, 0:1], w_tiles[0][0][:96, 0:1])
                nc.vector.tensor_copy(s_oV[:, 1:2], w_tiles[5][1][:96, 0:1])
                nc.sync.dma_start(out=out[:, :], in_=s_oV)
                raise StopIteration
            if stage == 3:
                s_oZ = big.tile([NQ, E], f32, name="s_oZ")
                nc.vector.memset(s_oZ, 0.0)
                nc.vector.tensor_copy(s_oZ[:, 0:1], s_V[:96, 0, 0:1])
                nc.vector.tensor_copy(s_oZ[:, 1:2], w_tiles[5][1][:96, 0:1])
                nc.sync.dma_start(out=out[:, :], in_=s_oZ)
                raise StopIteration
            # mask -> den -> AV -> out
            pden = [psS.tile([1, 4 * NQ], f32, tag=t_, name=f"pden{t_}")
                    for t_ in ("C", "D")]
            pa = [psS.tile([DK, 4, NQ], f32, tag=t_, name=f"pa{t_}")
                  for t_ in ("A", "B")]
            for ti, t in enumerate((0, 1, 2, 3, 4, 5)):
                msl = s_maskT[:, t, :]
                mrep = bass.AP(tensor=msl.tensor, offset=msl.offset,
                               ap=[msl.ap[0], [0, 4]] + msl.ap[1:])
                for q in range(2):
                    wt = w_tiles[t][q]
                    eng = nc.vector if q == 0 else nc.gpsimd
                    eng.tensor_mul(wt, wt, mrep)
                    nc.tensor.matmul(pden[q], onesrow, wt,
                                     start=(ti == 0), stop=(ti == TCH - 1))
            for ti, t in enumerate((0, 1, 2, 3, 4, 5)):
                for q in range(2):
                    for j in range(4):
                        h = 2 * j + q
                        nc.tensor.matmul(
                            pa[q][:, j, :],
                            s_V[:, t, DK * h:DK * (h + 1)],
                            w_tiles[t][q][:, NQ * j:NQ * (j + 1)],
                            start=(ti == 0 and j == 0 and True),
                            stop=(ti == TCH - 1 and j == 3))
            s_rden = tiny.tile([1, 2, 4 * NQ], f32)
            s_rdenb = big.tile([DK, 2, 4 * NQ], f32)
            for q in range(2):
                nc.vector.reciprocal(s_rden[:, q, :], pden[q])
                nc.gpsimd.partition_broadcast(
                    s_rdenb[:, q, :], s_rden[0:1, q, :], channels=DK)
            s_attn2 = big.tile([128, KCH, NQ], bf16)
            for q in range(2):
                nc.vector.tensor_mul(
                    s_attn2[DK * q:DK * q + DK, :, :].rearrange(
                        "p a b -> p (a b)"),
                    pa[q][:, :, :].rearrange("p a b -> p (a b)"),
                    s_rdenb[:, q, :])
            po_a = psS.tile([NQ, E // 2], f32, tag="D", name="po_a")
            po_b = psS.tile([NQ, E // 2], f32, tag="C", name="po_b")
            s_out = big.tile([NQ, E], f32)
            for half, po in ((0, po_a), (1, po_b)):
                e0 = half * (E // 2)
                for j in range(KCH):
                    nc.tensor.matmul(po, s_attn2[:, j, :],
                                     s_wo2[:, j, e0:e0 + E // 2],
                                     start=(j == 0), stop=(j == KCH - 1))
                nc.vector.tensor_add(
                    s_out[:, e0:e0 + E // 2], po,
                    s_const[:NQ, C_BO2 + e0:C_BO2 + e0 + E // 2])
                eng_d = nc.sync if half == 0 else nc.scalar
                eng_d.dma_start(out=out[:, e0:e0 + E // 2],
                                in_=s_out[:, e0:e0 + E // 2])

    nc.finalize()
    return nc


_NC_CACHE = {}


def _get_nc():
    if "nc" not in _NC_CACHE:
        _NC_CACHE["nc"] = build_nc()
    return _NC_CACHE["nc"]


def prep_inputs(x, Wq, bq_, Wk, bk_, Wv, bv_, Wo, bo_, iq_W, iq_b, ik_W, ik_b,
                wp_W, wp_b):
    bf = ml_dtypes.bfloat16
    f32 = np.float32
    xf = np.ascontiguousarray(np.asarray(x).reshape(S, E).astype(f32))
    xT = np.ascontiguousarray(xf.T)

    const = np.zeros((128, CC), f32)
    const[:, C_IQB:C_IQB + 4] = np.asarray(iq_b, f32).reshape(4, 128).T
    bqh = np.asarray(bq_, np.float64).reshape(H, DK)  # [head, d]
    for m in range(KCH):
        for a in range(2):
            const[64 * a:64 * a + 64, C_BQH + m] = bqh[2 * m + a]
    const[:DK, C_IKB] = np.asarray(ik_b, f32)
    const[DK:128, C_IKB] = np.asarray(ik_b, f32)
    const[0, C_WPB:C_WPB + H] = np.asarray(wp_b, f32)
    const[0, C_RAMP:C_RAMP + S] = (-np.arange(S, dtype=np.float64)
                                   * RAMP_EPS).astype(f32)
    const[:NQ, C_COL16:C_COL16 + 16] = np.arange(16, dtype=f32)
    bo2 = (np.asarray(bv_, np.float64) @ np.asarray(Wo, np.float64)
           + np.asarray(bo_, np.float64)).astype(f32)
    const[:NQ, C_BO2:C_BO2 + E] = bo2
    const[0, C_ONES:C_ONES + NQ] = 1.0

    W3 = np.asarray(Wo, f32).reshape(H, DK, E)
    wo2 = np.zeros((128, KCH, E), f32)
    for j in range(KCH):
        for a in range(2):
            wo2[64 * a:64 * a + 64, j, :] = W3[2 * j + a]

    shared = {
        "xT": xT,
        "iqW": np.ascontiguousarray(iq_W, f32),
        "ikW": np.ascontiguousarray(
            np.concatenate([ik_W, ik_W], axis=1), f32),
        "wpW": np.ascontiguousarray(wp_W, f32),
        "xT16": np.ascontiguousarray(xT).astype(bf),
        "wq16": np.ascontiguousarray(Wq).astype(bf),
        "wk16": np.ascontiguousarray(Wk).astype(bf),
        "wv16": np.ascontiguousarray(Wv).astype(bf),
        "wo2": np.ascontiguousarray(wo2).astype(bf),
        "constf": const,
    }
    shared["idb"] = np.eye(NQ, dtype=np.float32).astype(bf)
    in_maps = []
    for c in range(NC):
        m = dict(shared)
        m["xTq"] = np.ascontiguousarray(xT[:, NQ * c:NQ * (c + 1)])
        in_maps.append(m)
    return in_maps


def kernel(**inputs):
    from concourse.bass_utils import run_bass_kernel_spmd
    nc = _get_nc()
    in_maps = prep_inputs(
        inputs["x"], inputs["Wq"], inputs["bq"], inputs["Wk"], inputs["bk"],
        inputs["Wv"], inputs["bv"], inputs["Wo"], inputs["bo"],
        inputs["iq_W"], inputs["iq_b"], inputs["ik_W"], inputs["ik_b"],
        inputs["wp_W"], inputs["wp_b"])
    res = run_bass_kernel_spmd(nc, in_maps, core_ids=list(range(NC)))
    outs = [res.results[c]["out"] for c in range(NC)]
    return np.concatenate(outs, axis=0)[None].astype(np.float32)


# revision 35
# speedup vs baseline: 2.0667x; 1.0092x over previous
"""DeepSeek sparse attention TRN2 kernel: 8-core query-parallel, v2.

Hardcoded for B=1, S=768, E=512, H=8, DK=64, TOPK=384, 8 cores.
  - Core c owns queries [96c, 96c+96). Output = host concat of per-core rows.
  - Indexer chain in fp32 (top-k set needs ~1e-6 score accuracy: a single
    flipped boundary key costs ~2e-2 output error).
  - Per-head score matmuls straight out of the projection PSUM slices;
    combine = relu*w (DVE STT) + running add-chain on GpSimd, seeded with
    the -t*2^-40 tie-break ramp via partition_broadcast.
  - Top-k via per-row threshold: 8 bisection steps (DVE is_ge counts),
    then exact top-16 fixup (max8 + match_replace + max8).
  - Attention = dense QK^T + multiplicative 0/1 mask, bf16. Softmax
    denominators via ones-row matmul; 1/den broadcast across partitions
    with gpsimd.partition_broadcast (no DRAM round trip).
  - bk dropped (softmax shift-invariance); bv folded into bo2 on host.
  - Inputs split across three DMA queues (SP / ACT / GpSimd) in k-chunks
    so PE starts ~1.3us in; all small constants ride in one packed DMA.
"""
import numpy as np
import ml_dtypes

S, E, H, DK = 768, 512, 8, 64
NQ = 96
NC = 8
KCH = 4            # 512/128 contraction chunks
TCH = 6            # 768/128 key chunks
TH = 384           # half of S; fp32 PSUM-bank-sized N
SCALING = 1.0 / np.sqrt(DK)
RAMP_EPS = float(2.0 ** -40)
R_ITERS = 8
BRK = 1.2
NEG = -1e30

# const pack column layout
C_IQB = 0          # [128, 4]
C_BQH = 4          # [128, 4]
C_IKB = 8          # [64, 1]
C_WPB = 9          # [1, 8]
C_RAMP = 17        # [1, 768]
C_COL16 = 785      # [96, 16]
C_BO2 = 801        # [96, 512]
C_ONES = 1313      # [1, 96]
CC = 1409


def build_nc(stage=99):
    import concourse.bass as bass
    import concourse.bacc as bacc
    from concourse import mybir
    from concourse.tile import TileContext

    f32 = mybir.dt.float32
    bf16 = mybir.dt.bfloat16
    u8 = mybir.dt.uint8
    AF = mybir.ActivationFunctionType
    OP = mybir.AluOpType

    nc = bacc.Bacc("TRN2", target_bir_lowering=False, debug=False)

    def din(name, shape, dt):
        return nc.dram_tensor(name, shape, dt, kind="ExternalInput")

    xT = din("xT", [E, S], f32)
    xTq = din("xTq", [E, NQ], f32)
    iqW = din("iqW", [E, E], f32)
    ikW = din("ikW", [E, 2 * DK], f32)
    wpW = din("wpW", [E, H], f32)
    xT16 = din("xT16", [E, S], bf16)
    wq16 = din("wq16", [E, E], bf16)
    wk16 = din("wk16", [E, E], bf16)
    wv16 = din("wv16", [E, E], bf16)
    wo2 = din("wo2", [128, KCH, E], bf16)
    constf = din("constf", [128, CC], f32)
    idb = din("idb", [NQ, NQ], bf16)
    out = nc.dram_tensor("out", [NQ, E], f32, kind="ExternalOutput")

    import contextlib
    with TileContext(nc) as tc:
      with contextlib.suppress(StopIteration):
        with tc.tile_pool(name="w1", bufs=1) as w1, \
             tc.tile_pool(name="big", bufs=1) as big, \
             tc.tile_pool(name="tiny", bufs=1) as tiny, \
             tc.tile_pool(name="psS", bufs=1, space="PSUM") as psS, \
             tc.tile_pool(name="psR", bufs=4, space="PSUM") as psR:

            # ---------------- SBUF tiles ----------------
            s_xT = w1.tile([128, KCH, S], f32)
            s_xTq = w1.tile([128, KCH, NQ], f32)
            s_xq16 = w1.tile([128, KCH, NQ], bf16)
            s_iqW = w1.tile([128, KCH, E], f32)
            s_ikW = w1.tile([128, KCH, 2 * DK], f32)
            s_wpW = w1.tile([128, KCH, H], f32)
            s_xT16 = w1.tile([128, KCH, S], bf16)
            s_wq = w1.tile([128, KCH, E], bf16)
            s_wk = w1.tile([128, KCH, E], bf16)
            s_wv = w1.tile([128, KCH, E], bf16)
            s_wo2 = w1.tile([128, KCH, E], bf16)
            s_const = w1.tile([128, CC], f32)
            s_idb = w1.tile([NQ, NQ], bf16)

            # ---------------- DMA: three queues ----------------
            # SP queue: xTq, xT chunks, then bf16 weights
            warm = tiny.tile([1, 1], f32)
            nc.vector.memset(warm, 0.0)
            warmR = tiny.tile([128, E], bf16, name="warmR")
            nc.vector.memset(warmR, 0.0)
            # SP queue: xTq, xT k0/k1, then bf16 weights
            nc.sync.dma_start(
                out=s_xTq, in_=xTq[:, :].rearrange("(k p) n -> p k n", p=128))
            for k in range(2):
                nc.sync.dma_start(
                    out=s_xT[:, k, :], in_=xT[128 * k:128 * (k + 1), :])
            nc.sync.dma_start(
                out=s_wk, in_=wk16[:, :].rearrange("(k p) n -> p k n", p=128))
            nc.sync.dma_start(
                out=s_xT16, in_=xT16[:, :].rearrange("(k p) n -> p k n", p=128))
            nc.sync.dma_start(
                out=s_wq, in_=wq16[:, :].rearrange("(k p) n -> p k n", p=128))
            nc.sync.dma_start(
                out=s_wv, in_=wv16[:, :].rearrange("(k p) n -> p k n", p=128))
            nc.sync.dma_start(out=s_wo2, in_=wo2[:, :, :])
            # ACT queue: iqW k0, table-load warmup, iqW k1-k3
            nc.scalar.dma_start(out=s_iqW[:, 0, :], in_=iqW[0:128, :])
            nc.scalar.activation(out=warm, in_=warm, func=AF.Identity,
                                 bias=0.0)
            for k in range(1, KCH):
                nc.scalar.dma_start(
                    out=s_iqW[:, k, :], in_=iqW[128 * k:128 * (k + 1), :])
            # Pool queue: ikW, xT k2/k3, wpW, const
            nc.gpsimd.dma_start(
                out=s_ikW, in_=ikW[:, :].rearrange("(k p) n -> p k n", p=128))
            for k in range(2, KCH):
                nc.gpsimd.dma_start(
                    out=s_xT[:, k, :], in_=xT[128 * k:128 * (k + 1), :])
            nc.gpsimd.dma_start(
                out=s_wpW, in_=wpW[:, :].rearrange("(k p) n -> p k n", p=128))
            nc.gpsimd.dma_start(out=s_const, in_=constf[:, :])
            nc.gpsimd.dma_start(out=s_idb, in_=idb[:, :])

            onesrow = tiny.tile([128, 1], bf16)
            nc.vector.memset(onesrow, 1.0)
            negbig = tiny.tile([NQ, 1], f32, name="negbig")
            nc.vector.memset(negbig, NEG)

            nc.vector.tensor_copy(s_xq16.rearrange("p k n -> p (k n)"),
                                  s_xTq.rearrange("p k n -> p (k n)"))

            # PE p-state warmup: cheap dummy matmuls keep the ramp anchored
            for i in range(5):
                pwm = psR.tile([H, E], f32, tag="ps", name="pwm")
                nc.tensor.matmul(pwm, warmR[:, 0:H], warmR,
                                 start=True, stop=True)

            # =========== INDEXER projections (fp32) ===========
            pq = [psR.tile([128, NQ], f32, tag="ps", name=f"pq{m}")
                  for m in range(KCH)]
            pk0 = psS.tile([128, TH], f32, tag="B")
            pk1 = psS.tile([128, TH], f32, tag="C")
            pw = psR.tile([NQ, H], f32, tag="ps", name="pw")
            for k in range(KCH):
                nc.tensor.matmul(pw, s_xTq[:, k, :], s_wpW[:, k, :],
                                 start=(k == 0), stop=False)
                for th, pk in ((0, pk0), (1, pk1)):
                    nc.tensor.matmul(pk, s_ikW[:, k, :],
                                     s_xT[:, k, TH * th:TH * (th + 1)],
                                     start=(k == 0), stop=(k == KCH - 1))
            for m in range(KCH):
                for k in range(KCH):
                    nc.tensor.matmul(pq[m],
                                     s_iqW[:, k, 128 * m:128 * (m + 1)],
                                     s_xTq[:, k, :],
                                     start=(k == 0), stop=(k == KCH - 1))
            # wTq bias as a K=1 matmul: ones96^T @ wpb
            nc.tensor.matmul(pw, s_const[0:1, C_ONES:C_ONES + NQ],
                             s_const[0:1, C_WPB:C_WPB + H],
                             start=False, stop=True)

            s_qid = big.tile([128, KCH, NQ], f32)
            s_kidT = big.tile([128, 2, TH], f32)
            s_wTq = tiny.tile([NQ, H], f32)
            for m in range(KCH):
                nc.scalar.activation(out=s_qid[:, m, :], in_=pq[m],
                                     func=AF.Identity,
                                     bias=s_const[:, C_IQB + m:C_IQB + m + 1])
            for th, pk in ((0, pk0), (1, pk1)):
                nc.scalar.activation(out=s_kidT[:, th, :], in_=pk,
                                     func=AF.Identity,
                                     bias=s_const[:, C_IKB:C_IKB + 1])
            nc.vector.tensor_copy(s_wTq, pw)

            # combine accumulator seeded with the tie-break ramp
            s_ind = big.tile([NQ, S], f32)
            nc.gpsimd.partition_broadcast(
                s_ind, s_const[0:1, C_RAMP:C_RAMP + S], channels=NQ)

            # =========== per-head scores + combine ===========
            ws = [big.tile([NQ, 2, TH], f32, tag=f"ws{h}", name=f"ws{h}")
                  for h in range(H)]
            for h in range(H):
                p0 = DK * (h % 2)
                lhs = s_qid[p0:p0 + DK, h // 2, :]
                for th in range(2):
                    psc = psR.tile([NQ, TH], f32, tag="ps")
                    nc.tensor.matmul(psc, lhs, s_kidT[p0:p0 + DK, th, :],
                                     start=True, stop=True)
                    nc.vector.scalar_tensor_tensor(
                        out=ws[h][:, th, :], in0=psc, scalar=0.0,
                        in1=s_wTq[:, h:h + 1].to_broadcast([NQ, TH]),
                        op0=OP.max, op1=OP.mult)
                nc.gpsimd.tensor_add(
                    s_ind.rearrange("p n -> p n"),
                    s_ind.rearrange("p n -> p n"),
                    ws[h].rearrange("p a b -> p (a b)"))

            if stage == 1:
                s_oY = big.tile([NQ, E], f32, name="s_oY")
                nc.vector.memset(s_oY, 0.0)
                nc.vector.tensor_copy(s_oY[:, 0:2], s_ind[:, 0:2])
                nc.sync.dma_start(out=out[:, :], in_=s_oY)
                raise StopIteration
            # =========== TOPK threshold (bisection on DVE) ===========
            hp = tc.high_priority()
            hp.__enter__()
            rsum = tiny.tile([NQ, 1], f32, name="rsum")
            scr = big.tile([NQ, S], bf16, tag="scr")
            nc.vector.tensor_scalar(scr, s_ind, 0.0, None, op0=OP.add,
                                    op1=OP.add, accum_out=rsum)
            lo = tiny.tile([NQ, 1], f32)
            hi = tiny.tile([NQ, 1], f32)
            tmp = tiny.tile([NQ, 1], f32)
            mid = tiny.tile([NQ, 1], f32)
            cnt = tiny.tile([NQ, 1], f32)
            cmp = tiny.tile([NQ, 1], u8)
            ncmp = tiny.tile([NQ, 1], u8)
            nc.vector.tensor_scalar(lo, rsum, 1.0 / S, -BRK, op0=OP.mult,
                                    op1=OP.add)
            nc.vector.tensor_scalar(hi, rsum, 1.0 / S, BRK, op0=OP.mult,
                                    op1=OP.add)
            for r in range(R_ITERS):
                nc.vector.tensor_add(tmp, lo, hi)
                nc.vector.tensor_scalar_mul(mid, tmp, 0.5)
                nc.vector.tensor_scalar(scr, s_ind, mid, None, op0=OP.is_ge,
                                        op1=OP.add, accum_out=cnt)
                nc.vector.tensor_scalar(cmp, cnt, 384.0, None, op0=OP.is_ge)
                nc.vector.tensor_scalar(ncmp, cnt, 384.0, None, op0=OP.is_lt)
                nc.vector.copy_predicated(lo, cmp, mid)
                nc.vector.copy_predicated(hi, ncmp, mid)

            # exact count at hi; in-bracket top-16
            c_hi = tiny.tile([NQ, 1], f32)
            nc.vector.tensor_scalar(scr, s_ind, hi, None, op0=OP.is_ge,
                                    op1=OP.add, accum_out=c_hi)
            hicut = big.tile([NQ, S], f32, tag="hicut")
            nc.vector.scalar_tensor_tensor(
                out=hicut, in0=s_ind, scalar=hi,
                in1=negbig.to_broadcast([NQ, S]), op0=OP.is_ge, op1=OP.mult)
            mlo = big.tile([NQ, S], f32, tag="mlo")
            nc.vector.tensor_add(mlo[:, :TH], hicut[:, :TH], s_ind[:, :TH])
            nc.gpsimd.tensor_add(mlo[:, TH:], hicut[:, TH:], s_ind[:, TH:])
            m16 = tiny.tile([NQ, 16], f32)
            mlo2 = big.tile([NQ, S], f32, tag="mlo2")
            nc.vector.max(out=m16[:, 0:8], in_=mlo)
            nc.vector.match_replace(out=mlo2, in_to_replace=m16[:, 0:8],
                                    in_values=mlo, imm_value=NEG)
            nc.vector.max(out=m16[:, 8:16], in_=mlo2)
            need_m1 = tiny.tile([NQ, 1], f32)
            nc.vector.tensor_scalar(need_m1, c_hi, -1.0, 383.0, op0=OP.mult,
                                    op1=OP.add)
            oh = tiny.tile([NQ, 16], f32)
            tstar = tiny.tile([NQ, 1], f32)
            nc.vector.tensor_scalar(oh, s_const[:NQ, C_COL16:C_COL16 + 16],
                                    need_m1, None, op0=OP.is_equal)
            nc.vector.scalar_tensor_tensor(out=oh, in0=m16, scalar=1.0,
                                           in1=oh, op0=OP.mult, op1=OP.mult,
                                           accum_out=tstar)
            mask01 = big.tile([NQ, S], bf16, tag="mask01")
            nc.vector.tensor_scalar(mask01, s_ind, tstar, None, op0=OP.is_ge)
            s_maskT = big.tile([128, TCH, NQ], bf16)
            pmT = psS.tile([128, TCH, NQ], bf16, tag="D", name="pmT")
            for t in range(TCH):
                nc.tensor.transpose(pmT[:, t, :],
                                    mask01[:, 128 * t:128 * (t + 1)], s_idb)
            nc.vector.tensor_copy(
                s_maskT.rearrange("p a b -> p (a b)"),
                pmT.rearrange("p a b -> p (a b)"))
            gate = tiny.tile([128, 1], bf16, name="gate")
            nc.vector.tensor_scalar(gate, s_maskT[:, 0, 0:1], 0.0, 1.0,
                                    op0=OP.mult, op1=OP.add)
            hp.__exit__(None, None, None)
            if stage == 2:
                s_oX = big.tile([NQ, E], f32, name="s_oX")
                nc.vector.memset(s_oX, 0.0)
                nc.vector.tensor_copy(s_oX[:, 0:1], s_maskT[:96, 0, 0:1])
                nc.sync.dma_start(out=out[:, :], in_=s_oX)
                raise StopIteration

            # =========== ATTENTION (bf16) ===========
            s_KT = big.tile([128, KCH, S], bf16)
            s_QT = big.tile([128, KCH, NQ], bf16)
            s_V = big.tile([128, TCH, E], bf16)
            for m in range(KCH):
                for th in range(2):
                    pk2 = psR.tile([128, TH], f32, tag="ps")
                    for k in range(KCH):
                        nc.tensor.matmul(pk2,
                                         s_wk[:, k, 128 * m:128 * (m + 1)],
                                         s_xT16[:, k, TH * th:TH * (th + 1)],
                                         start=(k == 0), stop=(k == KCH - 1))
                    if m >= 2:
                        nc.vector.tensor_copy(
                            s_KT[:, m, TH * th:TH * (th + 1)], pk2)
                    else:
                        nc.scalar.copy(s_KT[:, m, TH * th:TH * (th + 1)], pk2)
            for m in range(KCH):
                pq2 = psR.tile([128, NQ], f32, tag="ps")
                for k in range(KCH):
                    nc.tensor.matmul(pq2, s_wq[:, k, 128 * m:128 * (m + 1)],
                                     s_xq16[:, k, :],
                                     start=(k == 0), stop=(k == KCH - 1))
                nc.scalar.activation(out=s_QT[:, m, :], in_=pq2,
                                     func=AF.Identity,
                                     bias=s_const[:, C_BQH + m:C_BQH + m + 1])

            if stage == 25:
                s_oW = big.tile([NQ, E], f32, name="s_oW")
                nc.vector.memset(s_oW, 0.0)
                nc.vector.tensor_copy(s_oW[:, 0:1], s_KT[:96, 0, 0:1])
                nc.vector.tensor_copy(s_oW[:, 1:2], s_QT[:96, 0, 0:1])
                nc.sync.dma_start(out=out[:, :], in_=s_oW)
                raise StopIteration
            # QK^T -> exp (q0 tiles first so exp starts as soon as KT m0/m1
            # land), V projection MMs interleaved to fill the exp-paced gaps
            w_tiles = [[big.tile([128, 4 * NQ], bf16, tag=f"wt_{t}_{q}",
                                 name=f"wt_{t}_{q}") for q in range(2)]
                       for t in range(TCH)]
            def emit_qk(t, q):
                psc2 = psR.tile([128, 4 * NQ], f32, tag="ps", name="psc2")
                p0 = DK * q
                for j in range(4):
                    nc.tensor.matmul(
                        psc2[:, NQ * j:NQ * (j + 1)],
                        s_KT[p0:p0 + DK, j, 128 * t:128 * (t + 1)],
                        s_QT[p0:p0 + DK, j, :],
                        start=True, stop=True)
                nc.scalar.activation(out=w_tiles[t][q], in_=psc2,
                                     func=AF.Exp, scale=SCALING)
            def emit_v(t):
                pv = psR.tile([128, E], f32, tag="ps", name="pv")
                for k in range(KCH):
                    nc.tensor.matmul(pv, s_xT16[:, k, 128 * t:128 * (t + 1)],
                                     s_wv[:, k, :],
                                     start=(k == 0), stop=(k == KCH - 1))
                nc.scalar.copy(s_V[:, t, :], pv)
            for t in range(TCH):
                emit_qk(t, 0)
                if stage != 27:
                    emit_v(t)
            for t in range(TCH):
                emit_qk(t, 1)

            if stage in (26, 27):
                s_oV = big.tile([NQ, E], f32, name="s_oV")
                nc.vector.memset(s_oV, 0.0)
                nc.vector.tensor_copy(s_oV[:, 0:1], w_tiles[0][0][:96, 0:1])
                nc.vector.tensor_copy(s_oV[:, 1:2], w_tiles[5][1][:96, 0:1])
                nc.sync.dma_start(out=out[:, :], in_=s_oV)
                raise StopIteration
            if stage == 3:
                s_oZ = big.tile([NQ, E], f32, name="s_oZ")
                nc.vector.memset(s_oZ, 0.0)
                nc.vector.tensor_copy(s_oZ[:, 0:1], s_V[:96, 0, 0:1])
                nc.vector.tensor_copy(s_oZ[:, 1:2], w_tiles[5][1][:96, 0:1])
                nc.sync.dma_start(out=out[:, :], in_=s_oZ)
                raise StopIteration
            # mask -> den -> AV -> out
            pden = [psS.tile([1, 4 * NQ], f32, tag=t_, name=f"pden{t_}")
                    for t_ in ("C", "D")]
            pa = [psS.tile([DK, 4, NQ], f32, tag=t_, name=f"pa{t_}")
                  for t_ in ("A", "B")]
            for ti, t in enumerate((0, 1, 2, 3, 4, 5)):
                msl = s_maskT[:, t, :]
                mrep = bass.AP(tensor=msl.tensor, offset=msl.offset,
                               ap=[msl.ap[0], [0, 4]] + msl.ap[1:])
                for q in range(2):
                    wt = w_tiles[t][q]
                    eng = nc.vector if q == 0 else nc.gpsimd
                    eng.tensor_mul(wt, wt, mrep)
                    nc.tensor.matmul(pden[q], onesrow, wt,
                                     start=(ti == 0), stop=(ti == TCH - 1))
            for ti, t in enumerate((0, 1, 2, 3, 4, 5)):
                for q in range(2):
                    for j in range(4):
                        h = 2 * j + q
                        nc.tensor.matmul(
                            pa[q][:, j, :],
                            s_V[:, t, DK * h:DK * (h + 1)],
                            w_tiles[t][q][:, NQ * j:NQ * (j + 1)],
                            start=(ti == 0 and j == 0 and True),
                            stop=(ti == TCH - 1 and j == 3))
            s_rden = tiny.tile([1, 2, 4 * NQ], f32)
            s_rdenb = big.tile([DK, 2, 4 * NQ], f32)
            for q in range(2):
                nc.vector.reciprocal(s_rden[:, q, :], pden[q])
                nc.gpsimd.partition_broadcast(
                    s_rdenb[:, q, :], s_rden[0:1, q, :], channels=DK)
            s_attn2 = big.tile([128, KCH, NQ], bf16)
            for q in range(2):
                nc.vector.tensor_mul(
                    s_attn2[DK * q:DK * q + DK, :, :].rearrange(
                        "p a b -> p (a b)"),
                    pa[q][:, :, :].rearrange("p a b -> p (a b)"),
                    s_rdenb[:, q, :])
            po_a = psS.tile([NQ, E // 2], f32, tag="D", name="po_a")
            po_b = psS.tile([NQ, E // 2], f32, tag="C", name="po_b")
            s_out = big.tile([NQ, E], f32)
            for half, po in ((0, po_a), (1, po_b)):
                e0 = half * (E // 2)
                for j in range(KCH):
                    nc.tensor.matmul(po, s_attn2[:, j, :],
                                     s_wo2[:, j, e0:e0 + E // 2],
                                     start=(j == 0), stop=(j == KCH - 1))
                nc.vector.tensor_add(
                    s_out[:, e0:e0 + E // 2], po,
                    s_const[:NQ, C_BO2 + e0:C_BO2 + e0 + E // 2])
                eng_d = nc.sync if half == 0 else nc.scalar
                eng_d.dma_start(out=out[:, e0:e0 + E // 2],
                                in_=s_out[:, e0:e0 + E // 2])

    nc.finalize()
    return nc


_NC_CACHE = {}


def _get_nc():
    if "nc" not in _NC_CACHE:
        _NC_CACHE["nc"] = build_nc()
    return _NC_CACHE["nc"]


def prep_inputs(x, Wq, bq_, Wk, bk_, Wv, bv_, Wo, bo_, iq_W, iq_b, ik_W, ik_b,
                wp_W, wp_b):
    bf = ml_dtypes.bfloat16
    f32 = np.float32
    xf = np.ascontiguousarray(np.asarray(x).reshape(S, E).astype(f32))
    xT = np.ascontiguousarray(xf.T)

    const = np.zeros((128, CC), f32)
    const[:, C_IQB:C_IQB + 4] = np.asarray(iq_b, f32).reshape(4, 128).T
    bqh = np.asarray(bq_, np.float64).reshape(H, DK)  # [head, d]
    for m in range(KCH):
        for a in range(2):
            const[64 * a:64 * a + 64, C_BQH + m] = bqh[2 * m + a]
    const[:DK, C_IKB] = np.asarray(ik_b, f32)
    const[DK:128, C_IKB] = np.asarray(ik_b, f32)
    const[0, C_WPB:C_WPB + H] = np.asarray(wp_b, f32)
    const[0, C_RAMP:C_RAMP + S] = (-np.arange(S, dtype=np.float64)
                                   * RAMP_EPS).astype(f32)
    const[:NQ, C_COL16:C_COL16 + 16] = np.arange(16, dtype=f32)
    bo2 = (np.asarray(bv_, np.float64) @ np.asarray(Wo, np.float64)
           + np.asarray(bo_, np.float64)).astype(f32)
    const[:NQ, C_BO2:C_BO2 + E] = bo2
    const[0, C_ONES:C_ONES + NQ] = 1.0

    W3 = np.asarray(Wo, f32).reshape(H, DK, E)
    wo2 = np.zeros((128, KCH, E), f32)
    for j in range(KCH):
        for a in range(2):
            wo2[64 * a:64 * a + 64, j, :] = W3[2 * j + a]

    shared = {
        "xT": xT,
        "iqW": np.ascontiguousarray(iq_W, f32),
        "ikW": np.ascontiguousarray(
            np.concatenate([ik_W, ik_W], axis=1), f32),
        "wpW": np.ascontiguousarray(wp_W, f32),
        "xT16": np.ascontiguousarray(xT).astype(bf),
        "wq16": np.ascontiguousarray(Wq).astype(bf),
        "wk16": np.ascontiguousarray(Wk).astype(bf),
        "wv16": np.ascontiguousarray(Wv).astype(bf),
        "wo2": np.ascontiguousarray(wo2).astype(bf),
        "constf": const,
    }
    shared["idb"] = np.eye(NQ, dtype=np.float32).astype(bf)
    in_maps = []
    for c in range(NC):
        m = dict(shared)
        m["xTq"] = np.ascontiguousarray(xT[:, NQ * c:NQ * (c + 1)])
        in_maps.append(m)
    return in_maps


def kernel(**inputs):
    from concourse.bass_utils import run_bass_kernel_spmd
    nc = _get_nc()
    in_maps = prep_inputs(
        inputs["x"], inputs["Wq"], inputs["bq"], inputs["Wk"], inputs["bk"],
        inputs["Wv"], inputs["bv"], inputs["Wo"], inputs["bo"],
        inputs["iq_W"], inputs["iq_b"], inputs["ik_W"], inputs["ik_b"],
        inputs["wp_W"], inputs["wp_b"])
    res = run_bass_kernel_spmd(nc, in_maps, core_ids=list(range(NC)))
    outs = [res.results[c]["out"] for c in range(NC)]
    return np.concatenate(outs, axis=0)[None].astype(np.float32)
